# revision 20
# baseline (speedup 1.0000x reference)
"""Distributed GATv2 (2 layers + BN) Bass kernel for 8 trn2 NeuronCores.

Strategy: nodes partitioned by range across 8 cores (dst-ownership).
Each core:
  - computes BN1 stats partials -> AllReduce -> folds BN into Wl1/Wr1
  - computes xl1 = [bn(x)@Wl1s | 0.6*bn(x)@Wl1s@A1blk] for ALL nodes
    (bf16, local DRAM table, 264 cols) with chunked DMA
  - computes xr1T (feat-major, + folded biases) for its own nodes
  - edge phase L1: per 128-edge subtile (exact-degree bucketed, dst-
    grouped): indirect-DMA gather of xl1[src] rows, feat-major z via PE
    (transpose-accumulate + identity-matmul of an AP-broadcast xr),
    |z| on ACT; logits = 0.6*sl[src] + 0.4*att@|z| (the 0.6*att@xr[dst]
    term is constant per softmax group and cancels), exp with a global
    shift, transposed segment-sums numT/denT via static one-hot matmuls,
    feat-major epilogue -> h1T (bf16)
  - one AllGather of h1T (+BN2 stat partials packed in 2 extra rows)
  - BN2 fold, xl2 table for all nodes, edge phase L2 (same scheme)
Output per core: outT [16, NODES_PAD] f32; host unpermutes/concats.
"""
import sys
import numpy as np

sys.path.insert(0, "/opt/trn_rl_repo")

import concourse.bass as bass          # noqa: E402
import concourse.bacc as bacc          # noqa: E402
import concourse.tile as tile          # noqa: E402
from concourse import mybir            # noqa: E402
from concourse.bass_utils import run_bass_kernel_spmd  # noqa: E402
from concourse.masks import make_identity  # noqa: E402

F32 = mybir.dt.float32
BF = mybir.dt.bfloat16
I32 = mybir.dt.int32
NPBF = mybir.dt.np(BF)

NCORES = 8
HEADS = 8
BN_EPS = 1e-5
NEG_SLOPE = 0.2


class Cfg:
    def __init__(self, n_nodes, in_dim, hid, out, m1, m2):
        self.N = n_nodes
        self.IN = in_dim
        self.HID = hid
        self.OUT = out
        self.F1 = HEADS * hid
        self.F2 = HEADS * out
        self.F1E = self.F1 + 8       # xl1 row: features + 0.6*sl
        self.F2E = self.F2 + 8
        self.M1 = m1          # logit shift (softmax-invariant), layer 1
        self.M2 = m2
        self.NL = n_nodes // NCORES
        self.NT_PAD = ((n_nodes + 127) // 128) * 128
        self.KT = [min(128, in_dim), max(0, in_dim - 128)]  # K tiles for IN


def _schedule(deg_per_core):
    """Exact-degree bucketed, group-aligned subtile schedule (uniform
    across cores). Subtile = (b, node_start, nps): nps nodes of degree b,
    slot i occupying edge rows [i*b, i*b+b)."""
    maxdeg = max(int(d.max()) for d in deg_per_core)
    assert maxdeg <= 128
    counts = {}
    for b in range(1, maxdeg + 1):
        c = max(int((d == b).sum()) for d in deg_per_core)
        if c:
            counts[b] = c
    subtiles = []
    pos = 0
    for b in sorted(counts):
        nps_full = max(1, 128 // b)
        left = counts[b]
        while left > 0:
            room = 128 - (pos % 128)
            nps = min(nps_full, left, room)
            subtiles.append((b, pos, nps))
            pos += nps
            left -= nps
    while pos % 128:
        nps = 128 - (pos % 128)   # all-dummy filler, b=1
        subtiles.append((1, pos, nps))
        pos += nps
    return counts, pos, subtiles


def _preprocess(cfg, x, edge_index, W):
    N, NL = cfg.N, cfg.NL
    src = np.concatenate([edge_index[0], np.arange(N, dtype=np.int32)])
    dst = np.concatenate([edge_index[1], np.arange(N, dtype=np.int32)])
    order = np.argsort(dst, kind="stable")
    src, dst = src[order], dst[order]
    deg = np.bincount(dst, minlength=N)
    starts = np.zeros(N + 1, np.int64)
    np.cumsum(deg, out=starts[1:])
    # balanced node->core assignment: deal nodes round-robin by degree rank
    # so every core sees a near-identical degree multiset (minimizes the
    # max-over-cores bucket padding in the uniform SPMD schedule)
    import os as _os
    if _os.environ.get("RANGE_ASSIGN"):
        assign = [np.arange(c * NL, (c + 1) * NL) for c in range(NCORES)]
    else:
        by_deg = np.argsort(-deg, kind="stable")
        assign = [np.sort(by_deg[c::NCORES]) for c in range(NCORES)]
    deg_pc = [deg[assign[c]] for c in range(NCORES)]
    counts, NODES_PAD, subtiles = _schedule(deg_pc)
    NSUB = len(subtiles)
    NG = NODES_PAD // 128

    # per-core: assign each core's degree-b nodes to the schedule's
    # degree-b slots in order; leftover slots are dummies (-1)
    proc = np.full((NCORES, NODES_PAD), -1, np.int64)   # proc pos -> local node
    ppos = np.full((NCORES, NL), -1, np.int64)          # local node -> proc pos
    slot_pos = {}    # b -> list of node positions, schedule order
    for b, nstart, nps in subtiles:
        slot_pos.setdefault(b, []).extend(range(nstart, nstart + nps))
    for c in range(NCORES):
        d = deg_pc[c]
        for b in counts:
            ids = np.nonzero(d == b)[0]
            positions = slot_pos[b][:len(ids)]
            proc[c, positions] = ids
            ppos[c, ids] = positions
    store = np.empty(N, np.int64)
    for c in range(NCORES):
        store[assign[c]] = c * NODES_PAD + ppos[c]

    esrc1 = np.zeros((NCORES, 128, NSUB), np.int32)
    esrc2 = np.zeros((NCORES, 128, NSUB), np.int32)
    S_np = np.zeros((128, NODES_PAD), NPBF)      # static one-hot, shared
    for s, (b, nstart, nps) in enumerate(subtiles):
        for slot in range(nps):
            S_np[slot * b:(slot + 1) * b, nstart + slot] = 1.0
    for c in range(NCORES):
        for s, (b, nstart, nps) in enumerate(subtiles):
            for slot in range(nps):
                v = proc[c, nstart + slot]
                if v < 0:
                    continue   # dummy: S col has keep-alive rows anyway
                gv = int(assign[c][int(v)])
                e0 = starts[gv]
                p0 = slot * b
                esrc1[c, p0:p0 + b, s] = src[e0:e0 + b]
                esrc2[c, p0:p0 + b, s] = store[src[e0:e0 + b]]

    # dummy-slot rows gather row 0 (esrc already 0) and contribute to den
    # of the dummy node only; outputs for dummies are masked / ignored.

    xT = np.zeros((cfg.IN, cfg.NT_PAD), NPBF)
    xT[:, :N] = x.T.astype(NPBF)
    in_maps = []
    A1 = np.zeros((cfg.F1, HEADS), np.float32)
    for h in range(HEADS):
        A1[h * cfg.HID:(h + 1) * cfg.HID, h] = W["att1"][h]
    A2 = np.zeros((cfg.F2, HEADS), np.float32)
    for h in range(HEADS):
        A2[h * cfg.OUT:(h + 1) * cfg.OUT, h] = W["att2"][h]
    # xl table rhs: [Wl1 | 0.6*Wl1@A1blk]; logits matmul uses 0.4*A
    Wcat1 = np.concatenate([W["Wl1"], 0.6 * (W["Wl1"] @ A1)], 1).astype(np.float32)
    Wcat2 = np.concatenate([W["Wl2"], 0.6 * (W["Wl2"] @ A2)], 1).astype(np.float32)
    for c in range(NCORES):
        xTo = np.zeros((cfg.IN, NODES_PAD), NPBF)
        sel = proc[c] >= 0
        xTo[:, sel] = x[assign[c][proc[c][sel]]].T.astype(NPBF)
        mask32 = np.zeros((cfg.HID, NODES_PAD), NPBF)
        mask32[:, sel] = 1.0
        mean1m = np.zeros((128, cfg.HID), np.float32)
        mean1m[np.arange(128), np.arange(128) % cfg.HID] = 0.125
        mean2m = np.zeros((128, cfg.OUT), np.float32)
        mean2m[np.arange(128), np.arange(128) % cfg.OUT] = 0.125
        e1m = np.zeros((8, cfg.F1), np.float32)
        e1m[np.arange(cfg.F1) // cfg.HID, np.arange(cfg.F1)] = 1.0
        e2m = np.zeros((8, cfg.F2), np.float32)
        e2m[np.arange(cfg.F2) // cfg.OUT, np.arange(cfg.F2)] = 1.0
        in_maps.append({
            "xT": xT, "xTo": xTo, "mask32": mask32,
            "mean1m": mean1m, "mean2m": mean2m, "e1m": e1m, "e2m": e2m,
            "esrc1": np.ascontiguousarray(esrc1[c]),
            "esrc2": np.ascontiguousarray(esrc2[c]),
            "Stbl": S_np,
            "Wl1m": W["Wl1"].reshape(cfg.IN, HEADS, cfg.HID).mean(1).astype(np.float32),
            "Wl2m": W["Wl2"].reshape(cfg.HID, HEADS, cfg.OUT).mean(1).astype(np.float32),
            "Wcat1": Wcat1, "Wcat2": Wcat2,
            "Wr1": W["Wr1"].astype(np.float32),
            "Wr2": W["Wr2"].astype(np.float32),
            "A1": (0.4 * A1).astype(np.float32),
            "A2": (0.4 * A2).astype(np.float32),
            "gb1": np.stack([W["gamma1"], W["beta1"]], 1).astype(np.float32),
            "gb2": np.stack([W["gamma2"], W["beta2"]], 1).astype(np.float32),
            "b1c": W["b1"].reshape(-1, 1).astype(np.float32),
            "b2c": W["b2"].reshape(-1, 1).astype(np.float32),
        })
    meta = dict(NODES_PAD=NODES_PAD, NSUB=NSUB, NG=NG, subtiles=subtiles,
                proc=proc, assign=assign, in_maps=in_maps)
    return meta


def _build(cfg, meta):
    NODES_PAD, NSUB, NG = meta["NODES_PAD"], meta["NSUB"], meta["NG"]
    subtiles = meta["subtiles"]
    IN, F1, F2, HID, OUT = cfg.IN, cfg.F1, cfg.F2, cfg.HID, cfg.OUT
    F1E, F2E = cfg.F1E, cfg.F2E
    K0, K1 = cfg.KT
    NTP = cfg.NT_PAD
    C1, C2 = HID, OUT
    n_xl1_tiles = NTP // 128
    RECIP_N = 1.0 / cfg.N

    nc = bacc.Bacc("TRN2", target_bir_lowering=False, debug=False,
                   num_devices=NCORES)
    din = {}
    for name, shape, dt in [
            ("xT", [IN, NTP], BF), ("xTo", [IN, NODES_PAD], BF),
            ("mask32", [HID, NODES_PAD], BF),
            ("esrc1", [128, NSUB], I32), ("esrc2", [128, NSUB], I32),
            ("Stbl", [128, NODES_PAD], BF),
            ("Wcat1", [IN, F1E], F32), ("Wr1", [IN, F1], F32),
            ("Wcat2", [HID, F2E], F32), ("Wr2", [HID, F2], F32),
            ("A1", [F1, HEADS], F32), ("A2", [F2, HEADS], F32),
            ("gb1", [IN, 2], F32), ("gb2", [HID, 2], F32),
            ("b1c", [HID, 1], F32), ("b2c", [OUT, 1], F32),
            ("mean1m", [128, HID], F32), ("mean2m", [128, OUT], F32),
            ("Wl1m", [IN, HID], F32), ("Wl2m", [HID, OUT], F32),
            ("e1m", [8, F1], F32), ("e2m", [8, F2], F32)]:
        din[name] = nc.dram_tensor(name, shape, dt, kind="ExternalInput")
    outT = nc.dram_tensor("outT", [OUT, NODES_PAD], F32, kind="ExternalOutput")

    xl1_full = nc.dram_tensor("xl1_full", [NTP, F1E], BF)
    xl2_full = nc.dram_tensor("xl2_full", [NCORES * NODES_PAD, F2E], BF)
    st1_in = nc.dram_tensor("st1_in", [IN, 2], F32)
    bl1_d = nc.dram_tensor("bl1_d", [1, HID], F32)
    bl2_d = nc.dram_tensor("bl2_d", [1, OUT], F32)
    st1_out = nc.dram_tensor("st1_out", [IN, 2], F32)
    ag_in = nc.dram_tensor("ag_in", [HID + 2, NODES_PAD], BF)
    ag_out = nc.dram_tensor("ag_out", [NCORES * (HID + 2), NODES_PAD], BF,
                            addr_space="Shared")

    import contextlib
    with tile.TileContext(nc) as tc:
        ctx = contextlib.ExitStack()
        with ctx:
            cpool = ctx.enter_context(tc.tile_pool(name="const", bufs=1))
            rpool = ctx.enter_context(tc.tile_pool(name="resident", bufs=1))

            # ---------- constants ----------
            ident = cpool.tile([128, 128], BF)
            make_identity(nc, ident[:])
            ones_row = cpool.tile([1, 128], BF)
            nc.vector.memset(ones_row[:], 1.0)
            epsb = cpool.tile([128, 1], F32, tag="epsb")
            nc.vector.memset(epsb[:], BN_EPS)
            msh1 = cpool.tile([128, 1], F32, tag="msh1")
            nc.vector.memset(msh1[:], -float(cfg.M1))
            msh2 = cpool.tile([128, 1], F32, tag="msh2")
            nc.vector.memset(msh2[:], -float(cfg.M2))
            def const_bf(name, shape, tagn):
                tf = cpool.tile(shape, F32, tag=tagn + "f", name=tagn + "f")
                nc.sync.dma_start(out=tf[:], in_=din[name].ap())
                tb = cpool.tile(shape, BF, tag=tagn, name=tagn)
                nc.vector.tensor_copy(out=tb[:], in_=tf[:])
                return tb
            mean1 = const_bf("mean1m", [128, C1], "mean1")
            mean2 = const_bf("mean2m", [128, C2], "mean2")
            e1full = const_bf("e1m", [8, F1], "e1m")
            e2full = const_bf("e2m", [8, F2], "e2m")
            e1h = [e1full[:, h * 128:(h + 1) * 128] for h in range(F1 // 128)]
            e2h = [e2full[:, h * 128:(h + 1) * 128] for h in range(F2 // 128)]

            # index / one-hot tables resident
            esrc1_sb = rpool.tile([128, NSUB], I32)
            nc.sync.dma_start(out=esrc1_sb[:], in_=din["esrc1"].ap())
            esrc2_sb = rpool.tile([128, NSUB], I32)
            nc.sync.dma_start(out=esrc2_sb[:], in_=din["esrc2"].ap())
            S_sb = rpool.tile([128, NODES_PAD], BF)
            nc.sync.dma_start(out=S_sb[:], in_=din["Stbl"].ap())
            a1_sb = []
            for h in range(F1 // 128):
                t = rpool.tile([128, HEADS], BF, tag=f"a1_{h}")
                tf = rpool.tile([128, HEADS], F32, tag=f"a1f_{h}")
                nc.sync.dma_start(out=tf[:], in_=din["A1"].ap()[h * 128:(h + 1) * 128, :])
                nc.vector.tensor_copy(out=t[:], in_=tf[:])
                a1_sb.append(t)
            a2_sb = []
            for h in range(F2 // 128):
                t = rpool.tile([128, HEADS], BF, tag=f"a2_{h}")
                tf = rpool.tile([128, HEADS], F32, tag=f"a2f_{h}")
                nc.sync.dma_start(out=tf[:], in_=din["A2"].ap()[h * 128:(h + 1) * 128, :])
                nc.vector.tensor_copy(out=t[:], in_=tf[:])
                a2_sb.append(t)
            b1c_sb = rpool.tile([HID, 1], F32)
            nc.sync.dma_start(out=b1c_sb[:], in_=din["b1c"].ap())
            b2c_sb = rpool.tile([OUT, 1], F32)
            nc.sync.dma_start(out=b2c_sb[:], in_=din["b2c"].ap())
            mask32_sb = rpool.tile([HID, NODES_PAD], BF)
            nc.sync.dma_start(out=mask32_sb[:], in_=din["mask32"].ap())

            kt_sizes = [K0] + ([K1] if K1 else [])
            xTo_sb = []
            for ki, ks in enumerate(kt_sizes):
                t = rpool.tile([ks, NODES_PAD], BF, tag=f"xTo{ki}")
                nc.sync.dma_start(out=t[:], in_=din["xTo"].ap()[ki * 128:ki * 128 + ks, :])
                xTo_sb.append(t)

            # ---------- phase A: BN1 stats + AllReduce ----------
            with tc.tile_pool(name="pA", bufs=2) as pa, \
                 tc.tile_pool(name="pAs", bufs=1) as pas:
                for ki, ks in enumerate(kt_sizes):
                    st = pa.tile([ks, 2], F32, tag="st")
                    nc.vector.tensor_reduce(out=st[:, 0:1], in_=xTo_sb[ki][:],
                                            axis=mybir.AxisListType.X,
                                            op=mybir.AluOpType.add)
                    scr = pas.tile([ks, NODES_PAD], BF, tag="scr")
                    nc.scalar.activation(out=scr[:], in_=xTo_sb[ki][:],
                                         func=mybir.ActivationFunctionType.Square,
                                         accum_out=st[:, 1:2])
                    nc.sync.dma_start(out=st1_in.ap()[ki * 128:ki * 128 + ks, :],
                                      in_=st[:])
            nc.gpsimd.collective_compute(
                "AllReduce", mybir.AluOpType.add,
                ins=[st1_in.ap()], outs=[st1_out.ap()],
                replica_groups=[list(range(NCORES))])

            # fold stats -> s1, t1 and scaled weights
            s1_t, t1_t = [], []
            wl1s, wr1s = [], []
            with tc.tile_pool(name="pB", bufs=1) as pb:
                for ki, ks in enumerate(kt_sizes):
                    stg = pb.tile([ks, 2], F32, tag=f"stg{ki}")
                    nc.sync.dma_start(out=stg[:], in_=st1_out.ap()[ki * 128:ki * 128 + ks, :])
                    gb = pb.tile([ks, 2], F32, tag=f"gb{ki}")
                    nc.sync.dma_start(out=gb[:], in_=din["gb1"].ap()[ki * 128:ki * 128 + ks, :])
                    mean = pb.tile([ks, 1], F32, tag=f"mean{ki}")
                    nc.vector.tensor_scalar(out=mean[:], in0=stg[:, 0:1],
                                            scalar1=RECIP_N, scalar2=None,
                                            op0=mybir.AluOpType.mult)
                    q = pb.tile([ks, 1], F32, tag=f"q{ki}")
                    nc.vector.tensor_scalar(out=q[:], in0=stg[:, 1:2],
                                            scalar1=RECIP_N, scalar2=None,
                                            op0=mybir.AluOpType.mult)
                    m2 = pb.tile([ks, 1], F32, tag=f"m2{ki}")
                    nc.vector.tensor_tensor(out=m2[:], in0=mean[:], in1=mean[:],
                                            op=mybir.AluOpType.mult)
                    var = pb.tile([ks, 1], F32, tag=f"var{ki}")
                    nc.vector.tensor_tensor(out=var[:], in0=q[:], in1=m2[:],
                                            op=mybir.AluOpType.subtract)
                    sd = pb.tile([ks, 1], F32, tag=f"sd{ki}")
                    nc.scalar.activation(out=sd[:], in_=var[:],
                                         func=mybir.ActivationFunctionType.Sqrt,
                                         bias=epsb[:ks, :1])
                    rstd = pb.tile([ks, 1], F32, tag=f"rstd{ki}")
                    nc.vector.reciprocal(rstd[:], sd[:])
                    s1 = pb.tile([ks, 1], F32, tag=f"s1{ki}")
                    nc.vector.tensor_tensor(out=s1[:], in0=gb[:, 0:1], in1=rstd[:],
                                            op=mybir.AluOpType.mult)
                    ms = pb.tile([ks, 1], F32, tag=f"ms{ki}")
                    nc.vector.tensor_tensor(out=ms[:], in0=mean[:], in1=s1[:],
                                            op=mybir.AluOpType.mult)
                    t1 = pb.tile([ks, 1], F32, tag=f"t1{ki}")
                    nc.vector.tensor_tensor(out=t1[:], in0=gb[:, 1:2], in1=ms[:],
                                            op=mybir.AluOpType.subtract)
                    s1_t.append(s1)
                    t1_t.append(t1)
                    wcf = pb.tile([ks, F1E], F32, tag=f"wcf{ki}", name=f"wcf{ki}")
                    nc.sync.dma_start(out=wcf[:], in_=din["Wcat1"].ap()[ki * 128:ki * 128 + ks, :])
                    wcs = rpool.tile([ks, F1E], BF, tag=f"wcs{ki}")
                    nc.vector.tensor_scalar(out=wcs[:], in0=wcf[:],
                                            scalar1=s1[:, :1], scalar2=None,
                                            op0=mybir.AluOpType.mult)
                    wl1s.append(wcs)
                    wrf = pb.tile([ks, F1], F32, tag=f"wrf{ki}", name=f"wrf{ki}")
                    nc.sync.dma_start(out=wrf[:], in_=din["Wr1"].ap()[ki * 128:ki * 128 + ks, :])
                    wrs = rpool.tile([ks, F1], BF, tag=f"wrs{ki}")
                    nc.vector.tensor_scalar(out=wrs[:], in0=wrf[:],
                                            scalar1=s1[:, :1], scalar2=None,
                                            op0=mybir.AluOpType.mult)
                    wr1s.append(wrs)
                    # keep f32 sum for bias12
                    wsumf = pb.tile([ks, F1], F32, tag=f"wsumf{ki}",
                                    name=f"wsumf{ki}")
                    nc.vector.tensor_tensor(out=wsumf[:], in0=wcf[:, :F1],
                                            in1=wrf[:], op=mybir.AluOpType.add)
                    wsb = pb.tile([ks, F1], BF, tag=f"wsb{ki}", name=f"wsb{ki}")
                    nc.vector.tensor_copy(out=wsb[:], in_=wsumf[:])
                    if ki == 0:
                        wsum_t = [wsb]
                    else:
                        wsum_t.append(wsb)
                t1b = []
                for ki, ks in enumerate(kt_sizes):
                    tb = pb.tile([ks, 1], BF, tag=f"t1b{ki}")
                    nc.vector.tensor_copy(out=tb[:], in_=t1_t[ki][:])
                    t1b.append(tb)
                with tc.tile_pool(name="pBp", bufs=1, space="PSUM") as pbp:
                    bps = pbp.tile([1, F1], F32, space="PSUM")
                    for ki, ks in enumerate(kt_sizes):
                        nc.tensor.matmul(out=bps[:], lhsT=t1b[ki][:],
                                         rhs=wsum_t[ki][:],
                                         start=(ki == 0),
                                         stop=(ki == len(kt_sizes) - 1))
                    bias12 = rpool.tile([1, F1], BF)
                    nc.vector.tensor_copy(out=bias12[:], in_=bps[:])
                with tc.tile_pool(name="pBq", bufs=1, space="PSUM") as pbq:
                    blp = pbq.tile([1, HID], F32, space="PSUM")
                    for ki, ks in enumerate(kt_sizes):
                        wmf = pb.tile([ks, HID], F32, tag=f"wmf{ki}",
                                      name=f"wmf{ki}")
                        nc.sync.dma_start(
                            out=wmf[:],
                            in_=din["Wl1m"].ap()[ki * 128:ki * 128 + ks, :])
                        wmb = pb.tile([ks, HID], BF, tag=f"wmb{ki}",
                                      name=f"wmb{ki}")
                        nc.vector.tensor_copy(out=wmb[:], in_=wmf[:])
                        nc.tensor.matmul(out=blp[:], lhsT=t1b[ki][:], rhs=wmb[:],
                                         start=(ki == 0),
                                         stop=(ki == len(kt_sizes) - 1))
                    blr = pb.tile([1, HID], F32, tag="blr")
                    nc.vector.tensor_copy(out=blr[:], in_=blp[:])
                    nc.sync.dma_start(out=bl1_d.ap(), in_=blr[:])
                blc = rpool.tile([HID, 1], F32)
                nc.sync.dma_start(out=blc[:], in_=bl1_d.ap())
                bias1t = rpool.tile([HID, 1], F32)
                nc.vector.tensor_tensor(out=bias1t[:], in0=blc[:], in1=b1c_sb[:],
                                        op=mybir.AluOpType.add)

            # ---------- phase A2: xr1T resident (feat-major, own nodes) ----
            xr1T = []
            with tc.tile_pool(name="pC", bufs=2, space="PSUM") as pc:
                for g in range(NG):
                    halves = []
                    for h in range(F1 // 128):
                        ps = pc.tile([128, 128], F32, space="PSUM", tag="xr1p")
                        for ki, ks in enumerate(kt_sizes):
                            nc.tensor.matmul(
                                out=ps[:],
                                lhsT=wr1s[ki][:, h * 128:(h + 1) * 128],
                                rhs=xTo_sb[ki][:, g * 128:(g + 1) * 128],
                                start=(ki == 0), stop=False)
                        nc.tensor.matmul(out=ps[:],
                                         lhsT=bias12[:, h * 128:(h + 1) * 128],
                                         rhs=ones_row[:],
                                         start=False, stop=True)
                        t = rpool.tile([128, 128], BF, tag=f"xr1T_{g}_{h}")
                        if (g + h) % 2 == 0:
                            nc.vector.tensor_copy(out=t[:], in_=ps[:])
                        else:
                            nc.scalar.copy(out=t[:], in_=ps[:])
                        halves.append(t)
                    xr1T.append(halves)

            # ---------- phase A3: xl1_full table (chunked DMA) ----------
            CH = 16
            with tc.tile_pool(name="pD", bufs=2) as pd, \
                 tc.tile_pool(name="pDo", bufs=2) as pdo, \
                 tc.tile_pool(name="pDp", bufs=2, space="PSUM") as pdp:
                for c0 in range(0, n_xl1_tiles, CH):
                    nt = min(CH, n_xl1_tiles - c0)
                    lhs = []
                    for ki, ks in enumerate(kt_sizes):
                        lt = pd.tile([ks, CH * 128], BF, tag=f"xl1l{ki}")
                        nc.sync.dma_start(
                            out=lt[:, :nt * 128],
                            in_=din["xT"].ap()[ki * 128:ki * 128 + ks,
                                               c0 * 128:(c0 + nt) * 128])
                        lhs.append(lt)
                    ob = pdo.tile([128, CH * F1E], BF, tag="xl1o")
                    for j in range(nt):
                        ps = pdp.tile([128, F1E], F32, space="PSUM", tag="xl1p")
                        for ki, ks in enumerate(kt_sizes):
                            nc.tensor.matmul(
                                out=ps[:], lhsT=lhs[ki][:, j * 128:(j + 1) * 128],
                                rhs=wl1s[ki][:],
                                start=(ki == 0), stop=(ki == len(kt_sizes) - 1))
                        osl = ob[:, j * F1E:(j + 1) * F1E]
                        if j % 2 == 0:
                            nc.vector.tensor_copy(out=osl, in_=ps[:])
                        else:
                            nc.scalar.copy(out=osl, in_=ps[:])
                    nc.sync.dma_start(
                        out=xl1_full.ap()[c0 * 128:(c0 + nt) * 128, :]
                            .rearrange("(j p) f -> p j f", j=nt),
                        in_=ob[:, :nt * F1E].rearrange("p (j f) -> p j f", j=nt))

            # ---------- helper: edge phase ----------
            def edge_phase(F, FE, xfull, esrc_sb, ah_sb, eh_mats, meanm,
                           shift_ap, bias_col, out_cb, layer):
                nhalf = F // 128
                NB = 4 if nhalf == 2 else 8      # subtiles per batch
                groups = {}
                for s, (b, nstart, nps) in enumerate(subtiles):
                    groups.setdefault(nstart // 128, []).append(
                        (s, b, nstart % 128, nps))
                with tc.tile_pool(name=f"ge{layer}", bufs=4) as gp, \
                     tc.tile_pool(name=f"gz{layer}", bufs=2, space="PSUM") as gz, \
                     tc.tile_pool(name=f"gl{layer}", bufs=2, space="PSUM") as gl, \
                     tc.tile_pool(name=f"gn{layer}", bufs=2, space="PSUM") as gn, \
                     tc.tile_pool(name=f"gs{layer}", bufs=3) as gs:
                    for g in range(NG):
                        subs = groups[g]
                        nd = gn.tile([128, nhalf * 128 + 128], F32,
                                     space="PSUM", tag="numT")
                        numT = nd[:, :nhalf * 128]
                        denT = nd[:8, nhalf * 128:]
                        for b0 in range(0, len(subs), NB):
                            batch = subs[b0:b0 + NB]
                            nb = len(batch)
                            # z layout: column block (si*nhalf+h)*128
                            zts = gz.tile([128, nb * nhalf * 128], F32,
                                          space="PSUM", tag="zt", name="zt")
                            lg = gl.tile([128, nb * 8], F32, space="PSUM", tag="lg")
                            xls4 = gp.tile([128, NB * FE], BF, tag="xls")
                            for si, (s, b, noff, nps) in enumerate(batch):
                                nc.gpsimd.indirect_dma_start(
                                    out=xls4[:, si * FE:(si + 1) * FE],
                                    out_offset=None,
                                    in_=xfull.ap(),
                                    in_offset=bass.IndirectOffsetOnAxis(
                                        ap=esrc_sb[:, s:s + 1], axis=0))
                                for h in range(nhalf):
                                    zsl = zts[:, (si * nhalf + h) * 128:
                                              (si * nhalf + h + 1) * 128]
                                    nc.tensor.matmul(
                                        out=zsl,
                                        lhsT=xls4[:, si * FE + h * 128:
                                                  si * FE + (h + 1) * 128],
                                        rhs=ident[:],
                                        start=True, stop=False)
                                    xr_ap = (xr1T[g][h] if layer == 1 else xr2T[g])
                                    rep = xr_ap[:, noff:noff + nps, None] \
                                        .broadcast_to([128, nps, b])
                                    nc.tensor.matmul(
                                        out=zsl[:, :nps * b], lhsT=ident[:],
                                        rhs=rep, start=False, stop=True)
                                    if nps * b < 128:
                                        rep2 = xr_ap[:, noff:noff + 1, None] \
                                            .broadcast_to([128, 1, 128 - nps * b])
                                        nc.tensor.matmul(
                                            out=zsl[:, nps * b:],
                                            lhsT=ident[:],
                                            rhs=rep2, start=False, stop=True)
                            es = gs.tile([128, nb * nhalf * 128], BF, tag="es",
                                         name="es")
                            nc.scalar.activation(
                                out=es[:], in_=zts[:],
                                func=mybir.ActivationFunctionType.Abs)
                            for si, (s, b, noff, nps) in enumerate(batch):
                                lsl = lg[:, si * 8:(si + 1) * 8]
                                for h in range(nhalf):
                                    nc.tensor.matmul(
                                        out=lsl,
                                        lhsT=es[:, (si * nhalf + h) * 128:
                                                (si * nhalf + h + 1) * 128],
                                        rhs=ah_sb[h][:],
                                        start=(h == 0), stop=False)
                                nc.tensor.matmul(
                                    out=lsl, lhsT=ident[:],
                                    rhs=xls4[:, si * FE + F:si * FE + F + 8],
                                    start=False, stop=True)
                            w4 = gs.tile([128, nb * 8], BF, tag="w4")
                            nc.scalar.activation(
                                out=w4[:], in_=lg[:],
                                func=mybir.ActivationFunctionType.Exp,
                                bias=shift_ap[:, :1])
                            y4 = gp.tile([128, NB * F], BF, tag="y")
                            xls_f = xls4[:, :nb * FE].rearrange(
                                "p (s f) -> p s f", s=nb)[:, :, :F] \
                                .rearrange("p s (a b) -> p s a b", a=8)
                            wv = w4[:, :nb * 8, None].rearrange(
                                "p (s a) b -> p s a b", s=nb) \
                                .broadcast_to([128, nb, 8, F // 8])
                            nc.vector.tensor_tensor(
                                out=y4[:, :nb * F].rearrange(
                                    "p (s a b) -> p s a b", s=nb, a=8),
                                in0=xls_f, in1=wv, op=mybir.AluOpType.mult)
                            for si, (s, b, noff, nps) in enumerate(batch):
                                S_ap = S_sb[:, g * 128 + noff:g * 128 + noff + nps]
                                for h in range(nhalf):
                                    nc.tensor.matmul(
                                        out=numT[:, h * 128 + noff:h * 128 + noff + nps],
                                        lhsT=y4[:, si * F + h * 128:
                                                si * F + (h + 1) * 128],
                                        rhs=S_ap, start=True, stop=True)
                                nc.tensor.matmul(
                                    out=denT[0:8, noff:noff + nps],
                                    lhsT=w4[:, si * 8:(si + 1) * 8],
                                    rhs=S_ap, start=True, stop=True)
                        # ---- group epilogue ----
                        drec = gs.tile([8, 128], F32, tag="drec")
                        nc.vector.reciprocal(drec[:], denT[:])
                        drecb = gs.tile([8, 128], BF, tag="drecb")
                        nc.vector.tensor_copy(out=drecb[:], in_=drec[:])
                        onts = []
                        for h in range(nhalf):
                            rexp = gz.tile([128, 128], F32, space="PSUM",
                                           tag="zt")
                            nc.tensor.matmul(out=rexp[:], lhsT=eh_mats[h],
                                             rhs=drecb[:], start=True, stop=True)
                            rexpb = gs.tile([128, 128], BF, tag=f"rexpb{h}",
                                            name=f"rexpb{h}")
                            nc.scalar.copy(out=rexpb[:], in_=rexp[:])
                            ont = gs.tile([128, 128], BF, tag=f"ont{h}",
                                          name=f"ont{h}")
                            nc.vector.tensor_tensor(
                                out=ont[:], in0=numT[:, h * 128:(h + 1) * 128],
                                in1=rexpb[:], op=mybir.AluOpType.mult)
                            onts.append(ont)
                        cdim = C1 if layer == 1 else C2
                        ot = gl.tile([cdim, 128], F32, space="PSUM", tag="lg")
                        for h in range(nhalf):
                            nc.tensor.matmul(out=ot[:], lhsT=meanm[:, :cdim],
                                             rhs=onts[h][:], start=(h == 0),
                                             stop=(h == nhalf - 1))
                        out_cb(g, ot, bias_col)

            # ---------- phase B: layer-1 edges -> h1T ----------
            h1T = rpool.tile([HID, NODES_PAD], BF)
            oB = ctx.enter_context(tc.tile_pool(name="oB", bufs=2))

            def l1_out(g, ot_psum, bias_col):
                hrel = oB.tile([HID, 128], BF, tag="hrel")
                nc.scalar.activation(out=hrel[:], in_=ot_psum[:],
                                     func=mybir.ActivationFunctionType.Relu,
                                     bias=bias_col[:, :1])
                nc.vector.tensor_tensor(out=h1T[:, g * 128:(g + 1) * 128],
                                        in0=hrel[:],
                                        in1=mask32_sb[:, g * 128:(g + 1) * 128],
                                        op=mybir.AluOpType.mult)

            edge_phase(F1, F1E, xl1_full, esrc1_sb, a1_sb, e1h, mean1, msh1,
                       bias1t, l1_out, layer=1)

            # ---------- phase C: AllGather h1T + BN2 + xl2 + xr2T ----------
            with tc.tile_pool(name="pE", bufs=2) as pe:
                st2 = pe.tile([HID, 2], F32, tag="st2")
                nc.vector.tensor_reduce(out=st2[:, 0:1], in_=h1T[:],
                                        axis=mybir.AxisListType.X,
                                        op=mybir.AluOpType.add)
                scr2 = pe.tile([HID, NODES_PAD], BF, tag="scr2")
                nc.scalar.activation(out=scr2[:], in_=h1T[:],
                                     func=mybir.ActivationFunctionType.Square,
                                     accum_out=st2[:, 1:2])
                nc.sync.dma_start(out=ag_in.ap()[0:HID, :], in_=h1T[:])
                nc.sync.dma_start(out=ag_in.ap()[HID:HID + 1, 0:2 * HID],
                                  in_=st2[:, 0:1].bitcast(BF))
                nc.sync.dma_start(out=ag_in.ap()[HID + 1:HID + 2, 0:2 * HID],
                                  in_=st2[:, 1:2].bitcast(BF))
            nc.gpsimd.collective_compute(
                "AllGather", mybir.AluOpType.bypass,
                ins=[ag_in.ap()], outs=[ag_out.ap()],
                replica_groups=[list(range(NCORES))])

            with tc.tile_pool(name="pF", bufs=1) as pf:
                s2sum = pf.tile([HID, NCORES], F32, tag="s2sum")
                s2sq = pf.tile([HID, NCORES], F32, tag="s2sq")
                agf = ag_out.ap().bitcast(F32)
                for c in range(NCORES):
                    r = c * (HID + 2) + HID
                    nc.sync.dma_start(out=s2sum[:, c:c + 1],
                                      in_=agf[r:r + 1, 0:HID])
                    nc.sync.dma_start(out=s2sq[:, c:c + 1],
                                      in_=agf[r + 1:r + 2, 0:HID])
                stg = pf.tile([HID, 2], F32, tag="stg2")
                nc.vector.tensor_reduce(out=stg[:, 0:1], in_=s2sum[:],
                                        axis=mybir.AxisListType.X,
                                        op=mybir.AluOpType.add)
                nc.vector.tensor_reduce(out=stg[:, 1:2], in_=s2sq[:],
                                        axis=mybir.AxisListType.X,
                                        op=mybir.AluOpType.add)
                gb = pf.tile([HID, 2], F32, tag="gb2")
                nc.sync.dma_start(out=gb[:], in_=din["gb2"].ap())
                mean = pf.tile([HID, 1], F32, tag="mean2")
                nc.vector.tensor_scalar(out=mean[:], in0=stg[:, 0:1],
                                        scalar1=RECIP_N, scalar2=None,
                                        op0=mybir.AluOpType.mult)
                q = pf.tile([HID, 1], F32, tag="q2")
                nc.vector.tensor_scalar(out=q[:], in0=stg[:, 1:2],
                                        scalar1=RECIP_N, scalar2=None,
                                        op0=mybir.AluOpType.mult)
                m2 = pf.tile([HID, 1], F32, tag="m22")
                nc.vector.tensor_tensor(out=m2[:], in0=mean[:], in1=mean[:],
                                        op=mybir.AluOpType.mult)
                var = pf.tile([HID, 1], F32, tag="var2")
                nc.vector.tensor_tensor(out=var[:], in0=q[:], in1=m2[:],
                                        op=mybir.AluOpType.subtract)
                sd = pf.tile([HID, 1], F32, tag="sd2")
                nc.scalar.activation(out=sd[:], in_=var[:],
                                     func=mybir.ActivationFunctionType.Sqrt,
                                     bias=epsb[:HID, :1])
                rstd = pf.tile([HID, 1], F32, tag="rstd2")
                nc.vector.reciprocal(rstd[:], sd[:])
                s2 = pf.tile([HID, 1], F32, tag="s2")
                nc.vector.tensor_tensor(out=s2[:], in0=gb[:, 0:1], in1=rstd[:],
                                        op=mybir.AluOpType.mult)
                ms = pf.tile([HID, 1], F32, tag="ms2")
                nc.vector.tensor_tensor(out=ms[:], in0=mean[:], in1=s2[:],
                                        op=mybir.AluOpType.mult)
                t2 = pf.tile([HID, 1], F32, tag="t2")
                nc.vector.tensor_tensor(out=t2[:], in0=gb[:, 1:2], in1=ms[:],
                                        op=mybir.AluOpType.subtract)
                wc2f = pf.tile([HID, F2E], F32, tag="wc2f")
                nc.sync.dma_start(out=wc2f[:], in_=din["Wcat2"].ap())
                wr2f = pf.tile([HID, F2], F32, tag="wr2f")
                nc.sync.dma_start(out=wr2f[:], in_=din["Wr2"].ap())
                wl2s = rpool.tile([HID, F2E], BF)
                nc.vector.tensor_scalar(out=wl2s[:], in0=wc2f[:],
                                        scalar1=s2[:, :1], scalar2=None,
                                        op0=mybir.AluOpType.mult)
                wr2s = rpool.tile([HID, F2], BF)
                nc.vector.tensor_scalar(out=wr2s[:], in0=wr2f[:],
                                        scalar1=s2[:, :1], scalar2=None,
                                        op0=mybir.AluOpType.mult)
                t2b = pf.tile([HID, 1], BF, tag="t2b")
                nc.vector.tensor_copy(out=t2b[:], in_=t2[:])
                wsum = pf.tile([HID, F2], BF, tag="wsum")
                nc.vector.tensor_tensor(out=wsum[:], in0=wc2f[:, :F2],
                                        in1=wr2f[:], op=mybir.AluOpType.add)
                with tc.tile_pool(name="pFp", bufs=1, space="PSUM") as pfp:
                    bps = pfp.tile([1, F2], F32, space="PSUM")
                    nc.tensor.matmul(out=bps[:], lhsT=t2b[:], rhs=wsum[:],
                                     start=True, stop=True)
                    bias22 = rpool.tile([1, F2], BF)
                    nc.vector.tensor_copy(out=bias22[:], in_=bps[:])
                with tc.tile_pool(name="pFq", bufs=1, space="PSUM") as pfq:
                    wmf2 = pf.tile([HID, OUT], F32, tag="wmf2")
                    nc.sync.dma_start(out=wmf2[:], in_=din["Wl2m"].ap())
                    wmb2 = pf.tile([HID, OUT], BF, tag="wmb2")
                    nc.vector.tensor_copy(out=wmb2[:], in_=wmf2[:])
                    blp2 = pfq.tile([1, OUT], F32, space="PSUM")
                    nc.tensor.matmul(out=blp2[:], lhsT=t2b[:], rhs=wmb2[:],
                                     start=True, stop=True)
                    blr2 = pf.tile([1, OUT], F32, tag="blr2")
                    nc.vector.tensor_copy(out=blr2[:], in_=blp2[:])
                    nc.sync.dma_start(out=bl2_d.ap(), in_=blr2[:])
                blc2 = rpool.tile([OUT, 1], F32)
                nc.sync.dma_start(out=blc2[:], in_=bl2_d.ap())
                bias2t = rpool.tile([OUT, 1], F32)
                nc.vector.tensor_tensor(out=bias2t[:], in0=blc2[:], in1=b2c_sb[:],
                                        op=mybir.AluOpType.add)

            # xl2_full (chunked per source-core stripe)
            CH2 = 10
            with tc.tile_pool(name="pG", bufs=2) as pg, \
                 tc.tile_pool(name="pGo", bufs=2) as pgo, \
                 tc.tile_pool(name="pGp", bufs=2, space="PSUM") as pgp:
                for c_src in range(NCORES):
                    for t0 in range(0, NG, CH2):
                        nt = min(CH2, NG - t0)
                        lhs2 = pg.tile([HID, CH2 * 128], BF, tag="xl2l")
                        nc.sync.dma_start(
                            out=lhs2[:, :nt * 128],
                            in_=ag_out.ap()[c_src * (HID + 2):c_src * (HID + 2) + HID,
                                            t0 * 128:(t0 + nt) * 128])
                        ob = pgo.tile([128, CH2 * F2E], BF, tag="xl2o")
                        for j in range(nt):
                            ps = pgp.tile([128, F2E], F32, space="PSUM", tag="xl2p")
                            nc.tensor.matmul(out=ps[:],
                                             lhsT=lhs2[:, j * 128:(j + 1) * 128],
                                             rhs=wl2s[:], start=True, stop=True)
                            osl = ob[:, j * F2E:(j + 1) * F2E]
                            if j % 2 == 0:
                                nc.vector.tensor_copy(out=osl, in_=ps[:])
                            else:
                                nc.scalar.copy(out=osl, in_=ps[:])
                        r0 = c_src * NODES_PAD + t0 * 128
                        nc.sync.dma_start(
                            out=xl2_full.ap()[r0:r0 + nt * 128, :]
                                .rearrange("(j p) f -> p j f", j=nt),
                            in_=ob[:, :nt * F2E].rearrange("p (j f) -> p j f",
                                                           j=nt))
            # xr2T resident
            xr2T = []
            with tc.tile_pool(name="pH", bufs=2, space="PSUM") as ph:
                for g in range(NG):
                    ps = ph.tile([128, 128], F32, space="PSUM", tag="xr2p")
                    nc.tensor.matmul(out=ps[:], lhsT=wr2s[:],
                                     rhs=h1T[:, g * 128:(g + 1) * 128],
                                     start=True, stop=False)
                    nc.tensor.matmul(out=ps[:], lhsT=bias22[:], rhs=ones_row[:],
                                     start=False, stop=True)
                    t = rpool.tile([128, 128], BF, tag=f"xr2T_{g}")
                    if g % 2 == 0:
                        nc.vector.tensor_copy(out=t[:], in_=ps[:])
                    else:
                        nc.scalar.copy(out=t[:], in_=ps[:])
                    xr2T.append(t)

            # ---------- phase D: layer-2 edges -> outT ----------
            oD = ctx.enter_context(tc.tile_pool(name="oD", bufs=2))

            def l2_out(g, ot_psum, bias_col):
                ob = oD.tile([OUT, 128], F32, tag="ob")
                nc.scalar.activation(out=ob[:], in_=ot_psum[:],
                                     func=mybir.ActivationFunctionType.Identity,
                                     bias=bias_col[:, :1])
                nc.sync.dma_start(out=outT.ap()[:, g * 128:(g + 1) * 128],
                                  in_=ob[:])

            edge_phase(F2, F2E, xl2_full, esrc2_sb, a2_sb, e2h, mean2, msh2,
                       bias2t, l2_out, layer=2)

    nc.compile()
    return nc


_CACHE = {}


def _get_nc(cfg, meta):
    key = (cfg.N, cfg.IN, cfg.HID, cfg.OUT, meta["NSUB"], meta["NODES_PAD"],
           tuple(meta["subtiles"]))
    if key not in _CACHE:
        _CACHE[key] = _build(cfg, meta)
    return _CACHE[key]


def run(cfg, inputs):
    x = np.asarray(inputs["x"], np.float32)
    ei = np.asarray(inputs["edge_index"], np.int32)
    W = {k: np.asarray(inputs[k], np.float32) for k in
         ("Wl1", "Wr1", "att1", "b1", "gamma1", "beta1",
          "Wl2", "Wr2", "att2", "b2", "gamma2", "beta2")}
    meta = _preprocess(cfg, x, ei, W)
    nc = _get_nc(cfg, meta)
    res = run_bass_kernel_spmd(nc, meta["in_maps"], core_ids=list(range(NCORES)))
    out = np.empty((cfg.N, cfg.OUT), np.float32)
    proc = meta["proc"]
    for c in range(NCORES):
        oT = res.results[c]["outT"]      # [OUT, NODES_PAD]
        sel = proc[c] >= 0
        out[meta["assign"][c][proc[c][sel]]] = oT[:, sel].T
    return out, meta, nc


def kernel(**inputs):
    cfg = Cfg(50000, 200, 32, 16, m1=8.0, m2=10.0)
    out, _, _ = run(cfg, inputs)
    return out


# revision 21
# speedup vs baseline: 1.7884x; 1.7884x over previous
"""Distributed GATv2 (2 layers + BN) Bass kernel for 8 trn2 NeuronCores.

Strategy: nodes partitioned by range across 8 cores (dst-ownership).
Each core:
  - computes BN1 stats partials -> AllReduce -> folds BN into Wl1/Wr1
  - computes xl1 = [bn(x)@Wl1s | 0.6*bn(x)@Wl1s@A1blk] for ALL nodes
    (bf16, local DRAM table, 264 cols) with chunked DMA
  - computes xr1T (feat-major, + folded biases) for its own nodes
  - edge phase L1: per 128-edge subtile (exact-degree bucketed, dst-
    grouped): indirect-DMA gather of xl1[src] rows, feat-major z via PE
    (transpose-accumulate + identity-matmul of an AP-broadcast xr),
    |z| on ACT; logits = 0.6*sl[src] + 0.4*att@|z| (the 0.6*att@xr[dst]
    term is constant per softmax group and cancels), exp with a global
    shift, transposed segment-sums numT/denT via static one-hot matmuls,
    feat-major epilogue -> h1T (bf16)
  - one AllGather of h1T (+BN2 stat partials packed in 2 extra rows)
  - BN2 fold, xl2 table for all nodes, edge phase L2 (same scheme)
Output per core: outT [16, NODES_PAD] f32; host unpermutes/concats.
"""
import sys
import numpy as np

sys.path.insert(0, "/opt/trn_rl_repo")

import concourse.bass as bass          # noqa: E402
import concourse.bacc as bacc          # noqa: E402
import concourse.tile as tile          # noqa: E402
from concourse import mybir            # noqa: E402
from concourse.bass_utils import run_bass_kernel_spmd  # noqa: E402
from concourse.masks import make_identity  # noqa: E402

F32 = mybir.dt.float32
BF = mybir.dt.bfloat16
I32 = mybir.dt.int32
NPBF = mybir.dt.np(BF)

NCORES = 8
HEADS = 8
BN_EPS = 1e-5
NEG_SLOPE = 0.2


class Cfg:
    def __init__(self, n_nodes, in_dim, hid, out, m1, m2):
        self.N = n_nodes
        self.IN = in_dim
        self.HID = hid
        self.OUT = out
        self.F1 = HEADS * hid
        self.F2 = HEADS * out
        self.F1E = self.F1 + 8       # xl1 row: features + 0.6*sl
        self.F2E = self.F2 + 8
        self.M1 = m1          # logit shift (softmax-invariant), layer 1
        self.M2 = m2
        self.NL = n_nodes // NCORES
        self.NT_PAD = ((n_nodes + 127) // 128) * 128
        self.KT = [min(128, in_dim), max(0, in_dim - 128)]  # K tiles for IN


def _schedule(deg_per_core):
    """Exact-degree bucketed, group-aligned subtile schedule (uniform
    across cores). Subtile = (b, node_start, nps): nps nodes of degree b,
    slot i occupying edge rows [i*b, i*b+b)."""
    maxdeg = max(int(d.max()) for d in deg_per_core)
    assert maxdeg <= 128
    counts = {}
    for b in range(1, maxdeg + 1):
        c = max(int((d == b).sum()) for d in deg_per_core)
        if c:
            counts[b] = c
    subtiles = []
    pos = 0
    for b in sorted(counts):
        nps_full = max(1, 128 // b)
        left = counts[b]
        while left > 0:
            room = 128 - (pos % 128)
            nps = min(nps_full, left, room)
            subtiles.append((b, pos, nps))
            pos += nps
            left -= nps
    while pos % 128:
        nps = 128 - (pos % 128)   # all-dummy filler, b=1
        subtiles.append((1, pos, nps))
        pos += nps
    return counts, pos, subtiles


def _preprocess(cfg, x, edge_index, W):
    N, NL = cfg.N, cfg.NL
    src = np.concatenate([edge_index[0], np.arange(N, dtype=np.int32)])
    dst = np.concatenate([edge_index[1], np.arange(N, dtype=np.int32)])
    order = np.argsort(dst, kind="stable")
    src, dst = src[order], dst[order]
    deg = np.bincount(dst, minlength=N)
    starts = np.zeros(N + 1, np.int64)
    np.cumsum(deg, out=starts[1:])
    # balanced node->core assignment: deal nodes round-robin by degree rank
    # so every core sees a near-identical degree multiset (minimizes the
    # max-over-cores bucket padding in the uniform SPMD schedule)
    import os as _os
    if _os.environ.get("RANGE_ASSIGN"):
        assign = [np.arange(c * NL, (c + 1) * NL) for c in range(NCORES)]
    else:
        by_deg = np.argsort(-deg, kind="stable")
        assign = [np.sort(by_deg[c::NCORES]) for c in range(NCORES)]
    deg_pc = [deg[assign[c]] for c in range(NCORES)]
    counts, NODES_PAD, subtiles = _schedule(deg_pc)
    NSUB = len(subtiles)
    NG = NODES_PAD // 128

    # per-core: assign each core's degree-b nodes to the schedule's
    # degree-b slots in order; leftover slots are dummies (-1)
    proc = np.full((NCORES, NODES_PAD), -1, np.int64)   # proc pos -> local node
    ppos = np.full((NCORES, NL), -1, np.int64)          # local node -> proc pos
    slot_pos = {}    # b -> list of node positions, schedule order
    for b, nstart, nps in subtiles:
        slot_pos.setdefault(b, []).extend(range(nstart, nstart + nps))
    for c in range(NCORES):
        d = deg_pc[c]
        for b in counts:
            ids = np.nonzero(d == b)[0]
            positions = slot_pos[b][:len(ids)]
            proc[c, positions] = ids
            ppos[c, ids] = positions
    store = np.empty(N, np.int64)
    for c in range(NCORES):
        store[assign[c]] = c * NODES_PAD + ppos[c]

    esrc1 = np.zeros((NCORES, 128, NSUB), np.int32)
    esrc2 = np.zeros((NCORES, 128, NSUB), np.int32)
    S_np = np.zeros((128, NODES_PAD), NPBF)      # static one-hot, shared
    for s, (b, nstart, nps) in enumerate(subtiles):
        for slot in range(nps):
            S_np[slot * b:(slot + 1) * b, nstart + slot] = 1.0
    for c in range(NCORES):
        for s, (b, nstart, nps) in enumerate(subtiles):
            for slot in range(nps):
                v = proc[c, nstart + slot]
                if v < 0:
                    continue   # dummy: S col has keep-alive rows anyway
                gv = int(assign[c][int(v)])
                e0 = starts[gv]
                p0 = slot * b
                esrc1[c, p0:p0 + b, s] = src[e0:e0 + b]
                esrc2[c, p0:p0 + b, s] = store[src[e0:e0 + b]]

    # dummy-slot rows gather row 0 (esrc already 0) and contribute to den
    # of the dummy node only; outputs for dummies are masked / ignored.

    xT = np.zeros((cfg.IN, cfg.NT_PAD), NPBF)
    xT[:, :N] = x.T.astype(NPBF)
    in_maps = []
    A1 = np.zeros((cfg.F1, HEADS), np.float32)
    for h in range(HEADS):
        A1[h * cfg.HID:(h + 1) * cfg.HID, h] = W["att1"][h]
    A2 = np.zeros((cfg.F2, HEADS), np.float32)
    for h in range(HEADS):
        A2[h * cfg.OUT:(h + 1) * cfg.OUT, h] = W["att2"][h]
    # xl table rhs: [Wl1 | 0.6*Wl1@A1blk]; logits matmul uses 0.4*A
    Wcat1 = np.concatenate([W["Wl1"], 0.6 * (W["Wl1"] @ A1)], 1).astype(np.float32)
    Wcat2 = np.concatenate([W["Wl2"], 0.6 * (W["Wl2"] @ A2)], 1).astype(np.float32)
    for c in range(NCORES):
        xTo = np.zeros((cfg.IN, NODES_PAD), NPBF)
        sel = proc[c] >= 0
        xTo[:, sel] = x[assign[c][proc[c][sel]]].T.astype(NPBF)
        mask32 = np.zeros((cfg.HID, NODES_PAD), NPBF)
        mask32[:, sel] = 1.0
        mean1m = np.zeros((128, cfg.HID), np.float32)
        mean1m[np.arange(128), np.arange(128) % cfg.HID] = 0.125
        mean2m = np.zeros((128, cfg.OUT), np.float32)
        mean2m[np.arange(128), np.arange(128) % cfg.OUT] = 0.125
        e1m = np.zeros((8, cfg.F1), np.float32)
        e1m[np.arange(cfg.F1) // cfg.HID, np.arange(cfg.F1)] = 1.0
        e2m = np.zeros((8, cfg.F2), np.float32)
        e2m[np.arange(cfg.F2) // cfg.OUT, np.arange(cfg.F2)] = 1.0
        in_maps.append({
            "xT": xT, "xTo": xTo, "mask32": mask32,
            "mean1m": mean1m, "mean2m": mean2m, "e1m": e1m, "e2m": e2m,
            "esrc1": np.ascontiguousarray(esrc1[c]),
            "esrc2": np.ascontiguousarray(esrc2[c]),
            "Stbl": S_np,
            "Wl1m": W["Wl1"].reshape(cfg.IN, HEADS, cfg.HID).mean(1).astype(np.float32),
            "Wl2m": W["Wl2"].reshape(cfg.HID, HEADS, cfg.OUT).mean(1).astype(np.float32),
            "Wcat1": Wcat1, "Wcat2": Wcat2,
            "Wr1": W["Wr1"].astype(np.float32),
            "Wr2": W["Wr2"].astype(np.float32),
            "A1": (0.4 * A1).astype(np.float32),
            "A2": (0.4 * A2).astype(np.float32),
            "gb1": np.stack([W["gamma1"], W["beta1"]], 1).astype(np.float32),
            "gb2": np.stack([W["gamma2"], W["beta2"]], 1).astype(np.float32),
            "b1c": W["b1"].reshape(-1, 1).astype(np.float32),
            "b2c": W["b2"].reshape(-1, 1).astype(np.float32),
        })
    meta = dict(NODES_PAD=NODES_PAD, NSUB=NSUB, NG=NG, subtiles=subtiles,
                proc=proc, assign=assign, in_maps=in_maps)
    return meta


def _build(cfg, meta):
    NODES_PAD, NSUB, NG = meta["NODES_PAD"], meta["NSUB"], meta["NG"]
    subtiles = meta["subtiles"]
    IN, F1, F2, HID, OUT = cfg.IN, cfg.F1, cfg.F2, cfg.HID, cfg.OUT
    F1E, F2E = cfg.F1E, cfg.F2E
    K0, K1 = cfg.KT
    NTP = cfg.NT_PAD
    C1, C2 = HID, OUT
    n_xl1_tiles = NTP // 128
    RECIP_N = 1.0 / cfg.N

    nc = bacc.Bacc("TRN2", target_bir_lowering=False, debug=False,
                   num_devices=NCORES)
    din = {}
    for name, shape, dt in [
            ("xT", [IN, NTP], BF), ("xTo", [IN, NODES_PAD], BF),
            ("mask32", [HID, NODES_PAD], BF),
            ("esrc1", [128, NSUB], I32), ("esrc2", [128, NSUB], I32),
            ("Stbl", [128, NODES_PAD], BF),
            ("Wcat1", [IN, F1E], F32), ("Wr1", [IN, F1], F32),
            ("Wcat2", [HID, F2E], F32), ("Wr2", [HID, F2], F32),
            ("A1", [F1, HEADS], F32), ("A2", [F2, HEADS], F32),
            ("gb1", [IN, 2], F32), ("gb2", [HID, 2], F32),
            ("b1c", [HID, 1], F32), ("b2c", [OUT, 1], F32),
            ("mean1m", [128, HID], F32), ("mean2m", [128, OUT], F32),
            ("Wl1m", [IN, HID], F32), ("Wl2m", [HID, OUT], F32),
            ("e1m", [8, F1], F32), ("e2m", [8, F2], F32)]:
        din[name] = nc.dram_tensor(name, shape, dt, kind="ExternalInput")
    outT = nc.dram_tensor("outT", [OUT, NODES_PAD], F32, kind="ExternalOutput")

    xl1_full = nc.dram_tensor("xl1_full", [NTP, F1E], BF)
    xl2_full = nc.dram_tensor("xl2_full", [NCORES * NODES_PAD, F2E], BF)
    st1_in = nc.dram_tensor("st1_in", [IN, 2], F32)
    bl1_d = nc.dram_tensor("bl1_d", [1, HID], F32)
    bl2_d = nc.dram_tensor("bl2_d", [1, OUT], F32)
    st1_out = nc.dram_tensor("st1_out", [IN, 2], F32)
    ag_in = nc.dram_tensor("ag_in", [HID + 2, NODES_PAD], BF)
    ag_out = nc.dram_tensor("ag_out", [NCORES * (HID + 2), NODES_PAD], BF,
                            addr_space="Shared")

    import contextlib
    with tile.TileContext(nc) as tc:
        ctx = contextlib.ExitStack()
        with ctx:
            cpool = ctx.enter_context(tc.tile_pool(name="const", bufs=1))
            rpool = ctx.enter_context(tc.tile_pool(name="resident", bufs=1))

            # ---------- constants ----------
            ident = cpool.tile([128, 128], BF)
            make_identity(nc, ident[:])
            ones_row = cpool.tile([1, 128], BF)
            nc.vector.memset(ones_row[:], 1.0)
            epsb = cpool.tile([128, 1], F32, tag="epsb")
            nc.vector.memset(epsb[:], BN_EPS)
            msh1 = cpool.tile([128, 1], F32, tag="msh1")
            nc.vector.memset(msh1[:], -float(cfg.M1))
            msh2 = cpool.tile([128, 1], F32, tag="msh2")
            nc.vector.memset(msh2[:], -float(cfg.M2))
            def const_bf(name, shape, tagn):
                tf = cpool.tile(shape, F32, tag=tagn + "f", name=tagn + "f")
                nc.sync.dma_start(out=tf[:], in_=din[name].ap())
                tb = cpool.tile(shape, BF, tag=tagn, name=tagn)
                nc.vector.tensor_copy(out=tb[:], in_=tf[:])
                return tb
            mean1 = const_bf("mean1m", [128, C1], "mean1")
            mean2 = const_bf("mean2m", [128, C2], "mean2")
            e1full = const_bf("e1m", [8, F1], "e1m")
            e2full = const_bf("e2m", [8, F2], "e2m")
            e1h = [e1full[:, h * 128:(h + 1) * 128] for h in range(F1 // 128)]
            e2h = [e2full[:, h * 128:(h + 1) * 128] for h in range(F2 // 128)]

            # index / one-hot tables resident
            esrc1_sb = rpool.tile([128, NSUB], I32)
            nc.sync.dma_start(out=esrc1_sb[:], in_=din["esrc1"].ap())
            esrc2_sb = rpool.tile([128, NSUB], I32)
            nc.sync.dma_start(out=esrc2_sb[:], in_=din["esrc2"].ap())
            S_sb = rpool.tile([128, NODES_PAD], BF)
            nc.sync.dma_start(out=S_sb[:], in_=din["Stbl"].ap())
            a1_sb = []
            for h in range(F1 // 128):
                t = rpool.tile([128, HEADS], BF, tag=f"a1_{h}")
                tf = rpool.tile([128, HEADS], F32, tag=f"a1f_{h}")
                nc.sync.dma_start(out=tf[:], in_=din["A1"].ap()[h * 128:(h + 1) * 128, :])
                nc.vector.tensor_copy(out=t[:], in_=tf[:])
                a1_sb.append(t)
            a2_sb = []
            for h in range(F2 // 128):
                t = rpool.tile([128, HEADS], BF, tag=f"a2_{h}")
                tf = rpool.tile([128, HEADS], F32, tag=f"a2f_{h}")
                nc.sync.dma_start(out=tf[:], in_=din["A2"].ap()[h * 128:(h + 1) * 128, :])
                nc.vector.tensor_copy(out=t[:], in_=tf[:])
                a2_sb.append(t)
            b1c_sb = rpool.tile([HID, 1], F32)
            nc.sync.dma_start(out=b1c_sb[:], in_=din["b1c"].ap())
            b2c_sb = rpool.tile([OUT, 1], F32)
            nc.sync.dma_start(out=b2c_sb[:], in_=din["b2c"].ap())
            mask32_sb = rpool.tile([HID, NODES_PAD], BF)
            nc.sync.dma_start(out=mask32_sb[:], in_=din["mask32"].ap())

            kt_sizes = [K0] + ([K1] if K1 else [])
            xTo_sb = []
            for ki, ks in enumerate(kt_sizes):
                t = rpool.tile([ks, NODES_PAD], BF, tag=f"xTo{ki}")
                nc.sync.dma_start(out=t[:], in_=din["xTo"].ap()[ki * 128:ki * 128 + ks, :])
                xTo_sb.append(t)

            # ---------- phase A: BN1 stats + AllReduce ----------
            with tc.tile_pool(name="pA", bufs=2) as pa, \
                 tc.tile_pool(name="pAs", bufs=1) as pas:
                for ki, ks in enumerate(kt_sizes):
                    st = pa.tile([ks, 2], F32, tag="st")
                    nc.vector.tensor_reduce(out=st[:, 0:1], in_=xTo_sb[ki][:],
                                            axis=mybir.AxisListType.X,
                                            op=mybir.AluOpType.add)
                    scr = pas.tile([ks, NODES_PAD], BF, tag="scr")
                    nc.scalar.activation(out=scr[:], in_=xTo_sb[ki][:],
                                         func=mybir.ActivationFunctionType.Square,
                                         accum_out=st[:, 1:2])
                    nc.sync.dma_start(out=st1_in.ap()[ki * 128:ki * 128 + ks, :],
                                      in_=st[:])
            nc.gpsimd.collective_compute(
                "AllReduce", mybir.AluOpType.add,
                ins=[st1_in.ap()], outs=[st1_out.ap()],
                replica_groups=[list(range(NCORES))])

            # fold stats -> s1, t1 and scaled weights
            s1_t, t1_t = [], []
            wl1s, wr1s = [], []
            with tc.tile_pool(name="pB", bufs=1) as pb:
                for ki, ks in enumerate(kt_sizes):
                    stg = pb.tile([ks, 2], F32, tag=f"stg{ki}")
                    nc.sync.dma_start(out=stg[:], in_=st1_out.ap()[ki * 128:ki * 128 + ks, :])
                    gb = pb.tile([ks, 2], F32, tag=f"gb{ki}")
                    nc.sync.dma_start(out=gb[:], in_=din["gb1"].ap()[ki * 128:ki * 128 + ks, :])
                    mean = pb.tile([ks, 1], F32, tag=f"mean{ki}")
                    nc.vector.tensor_scalar(out=mean[:], in0=stg[:, 0:1],
                                            scalar1=RECIP_N, scalar2=None,
                                            op0=mybir.AluOpType.mult)
                    q = pb.tile([ks, 1], F32, tag=f"q{ki}")
                    nc.vector.tensor_scalar(out=q[:], in0=stg[:, 1:2],
                                            scalar1=RECIP_N, scalar2=None,
                                            op0=mybir.AluOpType.mult)
                    m2 = pb.tile([ks, 1], F32, tag=f"m2{ki}")
                    nc.vector.tensor_tensor(out=m2[:], in0=mean[:], in1=mean[:],
                                            op=mybir.AluOpType.mult)
                    var = pb.tile([ks, 1], F32, tag=f"var{ki}")
                    nc.vector.tensor_tensor(out=var[:], in0=q[:], in1=m2[:],
                                            op=mybir.AluOpType.subtract)
                    sd = pb.tile([ks, 1], F32, tag=f"sd{ki}")
                    nc.scalar.activation(out=sd[:], in_=var[:],
                                         func=mybir.ActivationFunctionType.Sqrt,
                                         bias=epsb[:ks, :1])
                    rstd = pb.tile([ks, 1], F32, tag=f"rstd{ki}")
                    nc.vector.reciprocal(rstd[:], sd[:])
                    s1 = pb.tile([ks, 1], F32, tag=f"s1{ki}")
                    nc.vector.tensor_tensor(out=s1[:], in0=gb[:, 0:1], in1=rstd[:],
                                            op=mybir.AluOpType.mult)
                    ms = pb.tile([ks, 1], F32, tag=f"ms{ki}")
                    nc.vector.tensor_tensor(out=ms[:], in0=mean[:], in1=s1[:],
                                            op=mybir.AluOpType.mult)
                    t1 = pb.tile([ks, 1], F32, tag=f"t1{ki}")
                    nc.vector.tensor_tensor(out=t1[:], in0=gb[:, 1:2], in1=ms[:],
                                            op=mybir.AluOpType.subtract)
                    s1_t.append(s1)
                    t1_t.append(t1)
                    wcf = pb.tile([ks, F1E], F32, tag=f"wcf{ki}", name=f"wcf{ki}")
                    nc.sync.dma_start(out=wcf[:], in_=din["Wcat1"].ap()[ki * 128:ki * 128 + ks, :])
                    wcs = rpool.tile([ks, F1E], BF, tag=f"wcs{ki}")
                    nc.vector.tensor_scalar(out=wcs[:], in0=wcf[:],
                                            scalar1=s1[:, :1], scalar2=None,
                                            op0=mybir.AluOpType.mult)
                    wl1s.append(wcs)
                    wrf = pb.tile([ks, F1], F32, tag=f"wrf{ki}", name=f"wrf{ki}")
                    nc.sync.dma_start(out=wrf[:], in_=din["Wr1"].ap()[ki * 128:ki * 128 + ks, :])
                    wrs = rpool.tile([ks, F1], BF, tag=f"wrs{ki}")
                    nc.vector.tensor_scalar(out=wrs[:], in0=wrf[:],
                                            scalar1=s1[:, :1], scalar2=None,
                                            op0=mybir.AluOpType.mult)
                    wr1s.append(wrs)
                    # keep f32 sum for bias12
                    wsumf = pb.tile([ks, F1], F32, tag=f"wsumf{ki}",
                                    name=f"wsumf{ki}")
                    nc.vector.tensor_tensor(out=wsumf[:], in0=wcf[:, :F1],
                                            in1=wrf[:], op=mybir.AluOpType.add)
                    wsb = pb.tile([ks, F1], BF, tag=f"wsb{ki}", name=f"wsb{ki}")
                    nc.vector.tensor_copy(out=wsb[:], in_=wsumf[:])
                    if ki == 0:
                        wsum_t = [wsb]
                    else:
                        wsum_t.append(wsb)
                t1b = []
                for ki, ks in enumerate(kt_sizes):
                    tb = pb.tile([ks, 1], BF, tag=f"t1b{ki}")
                    nc.vector.tensor_copy(out=tb[:], in_=t1_t[ki][:])
                    t1b.append(tb)
                with tc.tile_pool(name="pBp", bufs=1, space="PSUM") as pbp:
                    bps = pbp.tile([1, F1], F32, space="PSUM")
                    for ki, ks in enumerate(kt_sizes):
                        nc.tensor.matmul(out=bps[:], lhsT=t1b[ki][:],
                                         rhs=wsum_t[ki][:],
                                         start=(ki == 0),
                                         stop=(ki == len(kt_sizes) - 1))
                    bias12 = rpool.tile([1, F1], BF)
                    nc.vector.tensor_copy(out=bias12[:], in_=bps[:])
                with tc.tile_pool(name="pBq", bufs=1, space="PSUM") as pbq:
                    blp = pbq.tile([1, HID], F32, space="PSUM")
                    for ki, ks in enumerate(kt_sizes):
                        wmf = pb.tile([ks, HID], F32, tag=f"wmf{ki}",
                                      name=f"wmf{ki}")
                        nc.sync.dma_start(
                            out=wmf[:],
                            in_=din["Wl1m"].ap()[ki * 128:ki * 128 + ks, :])
                        wmb = pb.tile([ks, HID], BF, tag=f"wmb{ki}",
                                      name=f"wmb{ki}")
                        nc.vector.tensor_copy(out=wmb[:], in_=wmf[:])
                        nc.tensor.matmul(out=blp[:], lhsT=t1b[ki][:], rhs=wmb[:],
                                         start=(ki == 0),
                                         stop=(ki == len(kt_sizes) - 1))
                    blr = pb.tile([1, HID], F32, tag="blr")
                    nc.vector.tensor_copy(out=blr[:], in_=blp[:])
                    nc.sync.dma_start(out=bl1_d.ap(), in_=blr[:])
                blc = rpool.tile([HID, 1], F32)
                nc.sync.dma_start(out=blc[:], in_=bl1_d.ap())
                bias1t = rpool.tile([HID, 1], F32)
                nc.vector.tensor_tensor(out=bias1t[:], in0=blc[:], in1=b1c_sb[:],
                                        op=mybir.AluOpType.add)

            # ---------- phase A2: xr1T resident (feat-major, own nodes) ----
            xr1T = []
            with tc.tile_pool(name="pC", bufs=2, space="PSUM") as pc:
                for g in range(NG):
                    halves = []
                    for h in range(F1 // 128):
                        ps = pc.tile([128, 128], F32, space="PSUM", tag="xr1p")
                        for ki, ks in enumerate(kt_sizes):
                            nc.tensor.matmul(
                                out=ps[:],
                                lhsT=wr1s[ki][:, h * 128:(h + 1) * 128],
                                rhs=xTo_sb[ki][:, g * 128:(g + 1) * 128],
                                start=(ki == 0), stop=False)
                        nc.tensor.matmul(out=ps[:],
                                         lhsT=bias12[:, h * 128:(h + 1) * 128],
                                         rhs=ones_row[:],
                                         start=False, stop=True)
                        t = rpool.tile([128, 128], BF, tag=f"xr1T_{g}_{h}")
                        if (g + h) % 2 == 0:
                            nc.vector.tensor_copy(out=t[:], in_=ps[:])
                        else:
                            nc.scalar.copy(out=t[:], in_=ps[:])
                        halves.append(t)
                    xr1T.append(halves)

            # ---------- phase A3: xl1_full table (chunked DMA) ----------
            CH = 28
            with tc.tile_pool(name="pD", bufs=2) as pd, \
                 tc.tile_pool(name="pDo", bufs=2) as pdo, \
                 tc.tile_pool(name="pDp", bufs=2, space="PSUM") as pdp:
                for c0 in range(0, n_xl1_tiles, CH):
                    nt = min(CH, n_xl1_tiles - c0)
                    lhs = []
                    for ki, ks in enumerate(kt_sizes):
                        lt = pd.tile([ks, CH * 128], BF, tag=f"xl1l{ki}")
                        nc.sync.dma_start(
                            out=lt[:, :nt * 128],
                            in_=din["xT"].ap()[ki * 128:ki * 128 + ks,
                                               c0 * 128:(c0 + nt) * 128])
                        lhs.append(lt)
                    ob = pdo.tile([128, CH * F1E], BF, tag="xl1o")
                    for j in range(nt):
                        ps = pdp.tile([128, F1E], F32, space="PSUM", tag="xl1p")
                        for ki, ks in enumerate(kt_sizes):
                            nc.tensor.matmul(
                                out=ps[:], lhsT=lhs[ki][:, j * 128:(j + 1) * 128],
                                rhs=wl1s[ki][:],
                                start=(ki == 0), stop=(ki == len(kt_sizes) - 1))
                        osl = ob[:, j * F1E:(j + 1) * F1E]
                        if j % 2 == 0:
                            nc.vector.tensor_copy(out=osl, in_=ps[:])
                        else:
                            nc.scalar.copy(out=osl, in_=ps[:])
                    nc.sync.dma_start(
                        out=xl1_full.ap()[c0 * 128:(c0 + nt) * 128, :]
                            .rearrange("(j p) f -> p j f", j=nt),
                        in_=ob[:, :nt * F1E].rearrange("p (j f) -> p j f", j=nt))

            # ---------- helper: edge phase ----------
            def edge_phase(F, FE, xfull, esrc_sb, ah_sb, eh_mats, meanm,
                           shift_ap, bias_col, out_cb, layer):
                nhalf = F // 128
                NB = 4 if nhalf == 2 else 8      # subtiles per batch
                groups = {}
                for s, (b, nstart, nps) in enumerate(subtiles):
                    groups.setdefault(nstart // 128, []).append(
                        (s, b, nstart % 128, nps))
                with tc.tile_pool(name=f"ge{layer}", bufs=4) as gp, \
                     tc.tile_pool(name=f"gz{layer}", bufs=2, space="PSUM") as gz, \
                     tc.tile_pool(name=f"gl{layer}", bufs=2, space="PSUM") as gl, \
                     tc.tile_pool(name=f"gn{layer}", bufs=2, space="PSUM") as gn, \
                     tc.tile_pool(name=f"gs{layer}", bufs=3) as gs:
                    for g in range(NG):
                        subs = groups[g]
                        nd = gn.tile([128, nhalf * 128 + 128], F32,
                                     space="PSUM", tag="numT")
                        numT = nd[:, :nhalf * 128]
                        denT = nd[:8, nhalf * 128:]
                        for b0 in range(0, len(subs), NB):
                            batch = subs[b0:b0 + NB]
                            nb = len(batch)
                            # z layout: column block (si*nhalf+h)*128
                            zts = gz.tile([128, nb * nhalf * 128], F32,
                                          space="PSUM", tag="zt", name="zt")
                            lg = gl.tile([128, nb * 8], F32, space="PSUM", tag="lg")
                            xls4 = gp.tile([128, NB * FE], BF, tag="xls")
                            for si, (s, b, noff, nps) in enumerate(batch):
                                nc.gpsimd.indirect_dma_start(
                                    out=xls4[:, si * FE:(si + 1) * FE],
                                    out_offset=None,
                                    in_=xfull.ap(),
                                    in_offset=bass.IndirectOffsetOnAxis(
                                        ap=esrc_sb[:, s:s + 1], axis=0))
                                for h in range(nhalf):
                                    zsl = zts[:, (si * nhalf + h) * 128:
                                              (si * nhalf + h + 1) * 128]
                                    nc.tensor.matmul(
                                        out=zsl,
                                        lhsT=xls4[:, si * FE + h * 128:
                                                  si * FE + (h + 1) * 128],
                                        rhs=ident[:],
                                        start=True, stop=False)
                                    xr_ap = (xr1T[g][h] if layer == 1 else xr2T[g])
                                    rep = xr_ap[:, noff:noff + nps, None] \
                                        .broadcast_to([128, nps, b])
                                    nc.tensor.matmul(
                                        out=zsl[:, :nps * b], lhsT=ident[:],
                                        rhs=rep, start=False, stop=True)
                                    if nps * b < 128:
                                        rep2 = xr_ap[:, noff:noff + 1, None] \
                                            .broadcast_to([128, 1, 128 - nps * b])
                                        nc.tensor.matmul(
                                            out=zsl[:, nps * b:],
                                            lhsT=ident[:],
                                            rhs=rep2, start=False, stop=True)
                            es = gs.tile([128, nb * nhalf * 128], BF, tag="es",
                                         name="es")
                            nc.scalar.activation(
                                out=es[:], in_=zts[:],
                                func=mybir.ActivationFunctionType.Abs)
                            for si, (s, b, noff, nps) in enumerate(batch):
                                lsl = lg[:, si * 8:(si + 1) * 8]
                                for h in range(nhalf):
                                    nc.tensor.matmul(
                                        out=lsl,
                                        lhsT=es[:, (si * nhalf + h) * 128:
                                                (si * nhalf + h + 1) * 128],
                                        rhs=ah_sb[h][:],
                                        start=(h == 0), stop=False)
                                nc.tensor.matmul(
                                    out=lsl, lhsT=ident[:],
                                    rhs=xls4[:, si * FE + F:si * FE + F + 8],
                                    start=False, stop=True)
                            w4 = gs.tile([128, nb * 8], BF, tag="w4")
                            nc.scalar.activation(
                                out=w4[:], in_=lg[:],
                                func=mybir.ActivationFunctionType.Exp,
                                bias=shift_ap[:, :1])
                            y4 = gp.tile([128, NB * F], BF, tag="y")
                            xls_f = xls4[:, :nb * FE].rearrange(
                                "p (s f) -> p s f", s=nb)[:, :, :F] \
                                .rearrange("p s (a b) -> p s a b", a=8)
                            wv = w4[:, :nb * 8, None].rearrange(
                                "p (s a) b -> p s a b", s=nb) \
                                .broadcast_to([128, nb, 8, F // 8])
                            nc.vector.tensor_tensor(
                                out=y4[:, :nb * F].rearrange(
                                    "p (s a b) -> p s a b", s=nb, a=8),
                                in0=xls_f, in1=wv, op=mybir.AluOpType.mult)
                            for si, (s, b, noff, nps) in enumerate(batch):
                                S_ap = S_sb[:, g * 128 + noff:g * 128 + noff + nps]
                                for h in range(nhalf):
                                    nc.tensor.matmul(
                                        out=numT[:, h * 128 + noff:h * 128 + noff + nps],
                                        lhsT=y4[:, si * F + h * 128:
                                                si * F + (h + 1) * 128],
                                        rhs=S_ap, start=True, stop=True)
                                nc.tensor.matmul(
                                    out=denT[0:8, noff:noff + nps],
                                    lhsT=w4[:, si * 8:(si + 1) * 8],
                                    rhs=S_ap, start=True, stop=True)
                        # ---- group epilogue ----
                        drec = gs.tile([8, 128], F32, tag="drec")
                        nc.vector.reciprocal(drec[:], denT[:])
                        drecb = gs.tile([8, 128], BF, tag="drecb")
                        nc.vector.tensor_copy(out=drecb[:], in_=drec[:])
                        onts = []
                        for h in range(nhalf):
                            rexp = gz.tile([128, 128], F32, space="PSUM",
                                           tag="zt")
                            nc.tensor.matmul(out=rexp[:], lhsT=eh_mats[h],
                                             rhs=drecb[:], start=True, stop=True)
                            rexpb = gs.tile([128, 128], BF, tag=f"rexpb{h}",
                                            name=f"rexpb{h}")
                            nc.scalar.copy(out=rexpb[:], in_=rexp[:])
                            ont = gs.tile([128, 128], BF, tag=f"ont{h}",
                                          name=f"ont{h}")
                            nc.vector.tensor_tensor(
                                out=ont[:], in0=numT[:, h * 128:(h + 1) * 128],
                                in1=rexpb[:], op=mybir.AluOpType.mult)
                            onts.append(ont)
                        cdim = C1 if layer == 1 else C2
                        ot = gl.tile([cdim, 128], F32, space="PSUM", tag="lg")
                        for h in range(nhalf):
                            nc.tensor.matmul(out=ot[:], lhsT=meanm[:, :cdim],
                                             rhs=onts[h][:], start=(h == 0),
                                             stop=(h == nhalf - 1))
                        out_cb(g, ot, bias_col)

            # ---------- phase B: layer-1 edges -> h1T ----------
            h1T = rpool.tile([HID, NODES_PAD], BF)
            oB = ctx.enter_context(tc.tile_pool(name="oB", bufs=2))

            def l1_out(g, ot_psum, bias_col):
                hrel = oB.tile([HID, 128], BF, tag="hrel")
                nc.scalar.activation(out=hrel[:], in_=ot_psum[:],
                                     func=mybir.ActivationFunctionType.Relu,
                                     bias=bias_col[:, :1])
                nc.vector.tensor_tensor(out=h1T[:, g * 128:(g + 1) * 128],
                                        in0=hrel[:],
                                        in1=mask32_sb[:, g * 128:(g + 1) * 128],
                                        op=mybir.AluOpType.mult)

            edge_phase(F1, F1E, xl1_full, esrc1_sb, a1_sb, e1h, mean1, msh1,
                       bias1t, l1_out, layer=1)

            # ---------- phase C: AllGather h1T + BN2 + xl2 + xr2T ----------
            with tc.tile_pool(name="pE", bufs=2) as pe:
                st2 = pe.tile([HID, 2], F32, tag="st2")
                nc.vector.tensor_reduce(out=st2[:, 0:1], in_=h1T[:],
                                        axis=mybir.AxisListType.X,
                                        op=mybir.AluOpType.add)
                scr2 = pe.tile([HID, NODES_PAD], BF, tag="scr2")
                nc.scalar.activation(out=scr2[:], in_=h1T[:],
                                     func=mybir.ActivationFunctionType.Square,
                                     accum_out=st2[:, 1:2])
                nc.sync.dma_start(out=ag_in.ap()[0:HID, :], in_=h1T[:])
                nc.sync.dma_start(out=ag_in.ap()[HID:HID + 1, 0:2 * HID],
                                  in_=st2[:, 0:1].bitcast(BF))
                nc.sync.dma_start(out=ag_in.ap()[HID + 1:HID + 2, 0:2 * HID],
                                  in_=st2[:, 1:2].bitcast(BF))
            nc.gpsimd.collective_compute(
                "AllGather", mybir.AluOpType.bypass,
                ins=[ag_in.ap()], outs=[ag_out.ap()],
                replica_groups=[list(range(NCORES))])

            with tc.tile_pool(name="pF", bufs=1) as pf:
                s2sum = pf.tile([HID, NCORES], F32, tag="s2sum")
                s2sq = pf.tile([HID, NCORES], F32, tag="s2sq")
                agf = ag_out.ap().bitcast(F32)
                for c in range(NCORES):
                    r = c * (HID + 2) + HID
                    nc.sync.dma_start(out=s2sum[:, c:c + 1],
                                      in_=agf[r:r + 1, 0:HID])
                    nc.sync.dma_start(out=s2sq[:, c:c + 1],
                                      in_=agf[r + 1:r + 2, 0:HID])
                stg = pf.tile([HID, 2], F32, tag="stg2")
                nc.vector.tensor_reduce(out=stg[:, 0:1], in_=s2sum[:],
                                        axis=mybir.AxisListType.X,
                                        op=mybir.AluOpType.add)
                nc.vector.tensor_reduce(out=stg[:, 1:2], in_=s2sq[:],
                                        axis=mybir.AxisListType.X,
                                        op=mybir.AluOpType.add)
                gb = pf.tile([HID, 2], F32, tag="gb2")
                nc.sync.dma_start(out=gb[:], in_=din["gb2"].ap())
                mean = pf.tile([HID, 1], F32, tag="mean2")
                nc.vector.tensor_scalar(out=mean[:], in0=stg[:, 0:1],
                                        scalar1=RECIP_N, scalar2=None,
                                        op0=mybir.AluOpType.mult)
                q = pf.tile([HID, 1], F32, tag="q2")
                nc.vector.tensor_scalar(out=q[:], in0=stg[:, 1:2],
                                        scalar1=RECIP_N, scalar2=None,
                                        op0=mybir.AluOpType.mult)
                m2 = pf.tile([HID, 1], F32, tag="m22")
                nc.vector.tensor_tensor(out=m2[:], in0=mean[:], in1=mean[:],
                                        op=mybir.AluOpType.mult)
                var = pf.tile([HID, 1], F32, tag="var2")
                nc.vector.tensor_tensor(out=var[:], in0=q[:], in1=m2[:],
                                        op=mybir.AluOpType.subtract)
                sd = pf.tile([HID, 1], F32, tag="sd2")
                nc.scalar.activation(out=sd[:], in_=var[:],
                                     func=mybir.ActivationFunctionType.Sqrt,
                                     bias=epsb[:HID, :1])
                rstd = pf.tile([HID, 1], F32, tag="rstd2")
                nc.vector.reciprocal(rstd[:], sd[:])
                s2 = pf.tile([HID, 1], F32, tag="s2")
                nc.vector.tensor_tensor(out=s2[:], in0=gb[:, 0:1], in1=rstd[:],
                                        op=mybir.AluOpType.mult)
                ms = pf.tile([HID, 1], F32, tag="ms2")
                nc.vector.tensor_tensor(out=ms[:], in0=mean[:], in1=s2[:],
                                        op=mybir.AluOpType.mult)
                t2 = pf.tile([HID, 1], F32, tag="t2")
                nc.vector.tensor_tensor(out=t2[:], in0=gb[:, 1:2], in1=ms[:],
                                        op=mybir.AluOpType.subtract)
                wc2f = pf.tile([HID, F2E], F32, tag="wc2f")
                nc.sync.dma_start(out=wc2f[:], in_=din["Wcat2"].ap())
                wr2f = pf.tile([HID, F2], F32, tag="wr2f")
                nc.sync.dma_start(out=wr2f[:], in_=din["Wr2"].ap())
                wl2s = rpool.tile([HID, F2E], BF)
                nc.vector.tensor_scalar(out=wl2s[:], in0=wc2f[:],
                                        scalar1=s2[:, :1], scalar2=None,
                                        op0=mybir.AluOpType.mult)
                wr2s = rpool.tile([HID, F2], BF)
                nc.vector.tensor_scalar(out=wr2s[:], in0=wr2f[:],
                                        scalar1=s2[:, :1], scalar2=None,
                                        op0=mybir.AluOpType.mult)
                t2b = pf.tile([HID, 1], BF, tag="t2b")
                nc.vector.tensor_copy(out=t2b[:], in_=t2[:])
                wsum = pf.tile([HID, F2], BF, tag="wsum")
                nc.vector.tensor_tensor(out=wsum[:], in0=wc2f[:, :F2],
                                        in1=wr2f[:], op=mybir.AluOpType.add)
                with tc.tile_pool(name="pFp", bufs=1, space="PSUM") as pfp:
                    bps = pfp.tile([1, F2], F32, space="PSUM")
                    nc.tensor.matmul(out=bps[:], lhsT=t2b[:], rhs=wsum[:],
                                     start=True, stop=True)
                    bias22 = rpool.tile([1, F2], BF)
                    nc.vector.tensor_copy(out=bias22[:], in_=bps[:])
                with tc.tile_pool(name="pFq", bufs=1, space="PSUM") as pfq:
                    wmf2 = pf.tile([HID, OUT], F32, tag="wmf2")
                    nc.sync.dma_start(out=wmf2[:], in_=din["Wl2m"].ap())
                    wmb2 = pf.tile([HID, OUT], BF, tag="wmb2")
                    nc.vector.tensor_copy(out=wmb2[:], in_=wmf2[:])
                    blp2 = pfq.tile([1, OUT], F32, space="PSUM")
                    nc.tensor.matmul(out=blp2[:], lhsT=t2b[:], rhs=wmb2[:],
                                     start=True, stop=True)
                    blr2 = pf.tile([1, OUT], F32, tag="blr2")
                    nc.vector.tensor_copy(out=blr2[:], in_=blp2[:])
                    nc.sync.dma_start(out=bl2_d.ap(), in_=blr2[:])
                blc2 = rpool.tile([OUT, 1], F32)
                nc.sync.dma_start(out=blc2[:], in_=bl2_d.ap())
                bias2t = rpool.tile([OUT, 1], F32)
                nc.vector.tensor_tensor(out=bias2t[:], in0=blc2[:], in1=b2c_sb[:],
                                        op=mybir.AluOpType.add)

            # xl2_full (chunked per source-core stripe)
            CH2 = 25
            with tc.tile_pool(name="pG", bufs=2) as pg, \
                 tc.tile_pool(name="pGo", bufs=2) as pgo, \
                 tc.tile_pool(name="pGp", bufs=2, space="PSUM") as pgp:
                for c_src in range(NCORES):
                    for t0 in range(0, NG, CH2):
                        nt = min(CH2, NG - t0)
                        lhs2 = pg.tile([HID, CH2 * 128], BF, tag="xl2l")
                        nc.sync.dma_start(
                            out=lhs2[:, :nt * 128],
                            in_=ag_out.ap()[c_src * (HID + 2):c_src * (HID + 2) + HID,
                                            t0 * 128:(t0 + nt) * 128])
                        ob = pgo.tile([128, CH2 * F2E], BF, tag="xl2o")
                        for j in range(nt):
                            ps = pgp.tile([128, F2E], F32, space="PSUM", tag="xl2p")
                            nc.tensor.matmul(out=ps[:],
                                             lhsT=lhs2[:, j * 128:(j + 1) * 128],
                                             rhs=wl2s[:], start=True, stop=True)
                            osl = ob[:, j * F2E:(j + 1) * F2E]
                            if j % 2 == 0:
                                nc.vector.tensor_copy(out=osl, in_=ps[:])
                            else:
                                nc.scalar.copy(out=osl, in_=ps[:])
                        r0 = c_src * NODES_PAD + t0 * 128
                        nc.sync.dma_start(
                            out=xl2_full.ap()[r0:r0 + nt * 128, :]
                                .rearrange("(j p) f -> p j f", j=nt),
                            in_=ob[:, :nt * F2E].rearrange("p (j f) -> p j f",
                                                           j=nt))
            # xr2T resident
            xr2T = []
            with tc.tile_pool(name="pH", bufs=2, space="PSUM") as ph:
                for g in range(NG):
                    ps = ph.tile([128, 128], F32, space="PSUM", tag="xr2p")
                    nc.tensor.matmul(out=ps[:], lhsT=wr2s[:],
                                     rhs=h1T[:, g * 128:(g + 1) * 128],
                                     start=True, stop=False)
                    nc.tensor.matmul(out=ps[:], lhsT=bias22[:], rhs=ones_row[:],
                                     start=False, stop=True)
                    t = rpool.tile([128, 128], BF, tag=f"xr2T_{g}")
                    if g % 2 == 0:
                        nc.vector.tensor_copy(out=t[:], in_=ps[:])
                    else:
                        nc.scalar.copy(out=t[:], in_=ps[:])
                    xr2T.append(t)

            # ---------- phase D: layer-2 edges -> outT ----------
            oD = ctx.enter_context(tc.tile_pool(name="oD", bufs=2))

            def l2_out(g, ot_psum, bias_col):
                ob = oD.tile([OUT, 128], F32, tag="ob")
                nc.scalar.activation(out=ob[:], in_=ot_psum[:],
                                     func=mybir.ActivationFunctionType.Identity,
                                     bias=bias_col[:, :1])
                nc.sync.dma_start(out=outT.ap()[:, g * 128:(g + 1) * 128],
                                  in_=ob[:])

            edge_phase(F2, F2E, xl2_full, esrc2_sb, a2_sb, e2h, mean2, msh2,
                       bias2t, l2_out, layer=2)

    nc.compile()
    return nc


_CACHE = {}


def _get_nc(cfg, meta):
    key = (cfg.N, cfg.IN, cfg.HID, cfg.OUT, meta["NSUB"], meta["NODES_PAD"],
           tuple(meta["subtiles"]))
    if key not in _CACHE:
        _CACHE[key] = _build(cfg, meta)
    return _CACHE[key]


def run(cfg, inputs):
    x = np.asarray(inputs["x"], np.float32)
    ei = np.asarray(inputs["edge_index"], np.int32)
    W = {k: np.asarray(inputs[k], np.float32) for k in
         ("Wl1", "Wr1", "att1", "b1", "gamma1", "beta1",
          "Wl2", "Wr2", "att2", "b2", "gamma2", "beta2")}
    meta = _preprocess(cfg, x, ei, W)
    nc = _get_nc(cfg, meta)
    res = run_bass_kernel_spmd(nc, meta["in_maps"], core_ids=list(range(NCORES)))
    out = np.empty((cfg.N, cfg.OUT), np.float32)
    proc = meta["proc"]
    for c in range(NCORES):
        oT = res.results[c]["outT"]      # [OUT, NODES_PAD]
        sel = proc[c] >= 0
        out[meta["assign"][c][proc[c][sel]]] = oT[:, sel].T
    return out, meta, nc


def kernel(**inputs):
    cfg = Cfg(50000, 200, 32, 16, m1=8.0, m2=10.0)
    out, _, _ = run(cfg, inputs)
    return out


# revision 22
# speedup vs baseline: 3.0198x; 1.6885x over previous
"""Distributed GATv2 (2 layers + BN) Bass kernel for 8 trn2 NeuronCores.

Strategy: nodes partitioned by range across 8 cores (dst-ownership).
Each core:
  - computes BN1 stats partials -> AllReduce -> folds BN into Wl1/Wr1
  - computes xl1 = [bn(x)@Wl1s | 0.6*bn(x)@Wl1s@A1blk] for ALL nodes
    (bf16, local DRAM table, 264 cols) with chunked DMA
  - computes xr1T (feat-major, + folded biases) for its own nodes
  - edge phase L1: per 128-edge subtile (exact-degree bucketed, dst-
    grouped): indirect-DMA gather of xl1[src] rows, feat-major z via PE
    (transpose-accumulate + identity-matmul of an AP-broadcast xr),
    |z| on ACT; logits = 0.6*sl[src] + 0.4*att@|z| (the 0.6*att@xr[dst]
    term is constant per softmax group and cancels), exp with a global
    shift, transposed segment-sums numT/denT via static one-hot matmuls,
    feat-major epilogue -> h1T (bf16)
  - one AllGather of h1T (+BN2 stat partials packed in 2 extra rows)
  - BN2 fold, xl2 table for all nodes, edge phase L2 (same scheme)
Output per core: outT [16, NODES_PAD] f32; host unpermutes/concats.
"""
import sys
import numpy as np

sys.path.insert(0, "/opt/trn_rl_repo")

import concourse.bass as bass          # noqa: E402
import concourse.bacc as bacc          # noqa: E402
import concourse.tile as tile          # noqa: E402
from concourse import mybir            # noqa: E402
from concourse.bass_utils import run_bass_kernel_spmd  # noqa: E402
from concourse.masks import make_identity  # noqa: E402

F32 = mybir.dt.float32
BF = mybir.dt.bfloat16
I32 = mybir.dt.int32
NPBF = mybir.dt.np(BF)

NCORES = 8
HEADS = 8
BN_EPS = 1e-5
NEG_SLOPE = 0.2


class Cfg:
    def __init__(self, n_nodes, in_dim, hid, out, m1, m2):
        self.N = n_nodes
        self.IN = in_dim
        self.HID = hid
        self.OUT = out
        self.F1 = HEADS * hid
        self.F2 = HEADS * out
        self.F1E = self.F1 + 8       # xl1 row: features + 0.6*sl
        self.F2E = self.F2 + 8
        self.M1 = m1          # logit shift (softmax-invariant), layer 1
        self.M2 = m2
        self.NL = n_nodes // NCORES
        self.NT_PAD = ((n_nodes + 127) // 128) * 128
        self.KT = [min(128, in_dim), max(0, in_dim - 128)]  # K tiles for IN


def _schedule(deg_per_core):
    """Exact-degree bucketed, group-aligned subtile schedule (uniform
    across cores). Subtile = (b, node_start, nps): nps nodes of degree b,
    slot i occupying edge rows [i*b, i*b+b)."""
    maxdeg = max(int(d.max()) for d in deg_per_core)
    assert maxdeg <= 128
    counts = {}
    for b in range(1, maxdeg + 1):
        c = max(int((d == b).sum()) for d in deg_per_core)
        if c:
            counts[b] = c
    subtiles = []
    pos = 0
    for b in sorted(counts):
        nps_full = max(1, 128 // b)
        left = counts[b]
        while left > 0:
            room = 128 - (pos % 128)
            nps = min(nps_full, left, room)
            subtiles.append((b, pos, nps))
            pos += nps
            left -= nps
    while pos % 128:
        nps = 128 - (pos % 128)   # all-dummy filler, b=1
        subtiles.append((1, pos, nps))
        pos += nps
    return counts, pos, subtiles


def _preprocess(cfg, x, edge_index, W):
    N, NL = cfg.N, cfg.NL
    src = np.concatenate([edge_index[0], np.arange(N, dtype=np.int32)])
    dst = np.concatenate([edge_index[1], np.arange(N, dtype=np.int32)])
    order = np.argsort(dst, kind="stable")
    src, dst = src[order], dst[order]
    deg = np.bincount(dst, minlength=N)
    starts = np.zeros(N + 1, np.int64)
    np.cumsum(deg, out=starts[1:])
    # balanced node->core assignment: deal nodes round-robin by degree rank
    # so every core sees a near-identical degree multiset (minimizes the
    # max-over-cores bucket padding in the uniform SPMD schedule)
    import os as _os
    if _os.environ.get("RANGE_ASSIGN"):
        assign = [np.arange(c * NL, (c + 1) * NL) for c in range(NCORES)]
    else:
        by_deg = np.argsort(-deg, kind="stable")
        assign = [np.sort(by_deg[c::NCORES]) for c in range(NCORES)]
    deg_pc = [deg[assign[c]] for c in range(NCORES)]
    counts, NODES_PAD, subtiles = _schedule(deg_pc)
    NSUB = len(subtiles)
    NG = NODES_PAD // 128

    # per-core: assign each core's degree-b nodes to the schedule's
    # degree-b slots in order; leftover slots are dummies (-1)
    proc = np.full((NCORES, NODES_PAD), -1, np.int64)   # proc pos -> local node
    ppos = np.full((NCORES, NL), -1, np.int64)          # local node -> proc pos
    slot_pos = {}    # b -> list of node positions, schedule order
    for b, nstart, nps in subtiles:
        slot_pos.setdefault(b, []).extend(range(nstart, nstart + nps))
    for c in range(NCORES):
        d = deg_pc[c]
        for b in counts:
            ids = np.nonzero(d == b)[0]
            positions = slot_pos[b][:len(ids)]
            proc[c, positions] = ids
            ppos[c, ids] = positions
    store = np.empty(N, np.int64)
    for c in range(NCORES):
        store[assign[c]] = c * NODES_PAD + ppos[c]

    esrc1 = np.zeros((NCORES, 128, NSUB), np.int32)
    esrc2 = np.zeros((NCORES, 128, NSUB), np.int32)
    S_np = np.zeros((128, NODES_PAD), NPBF)      # static one-hot, shared
    for s, (b, nstart, nps) in enumerate(subtiles):
        for slot in range(nps):
            S_np[slot * b:(slot + 1) * b, nstart + slot] = 1.0
    for c in range(NCORES):
        for s, (b, nstart, nps) in enumerate(subtiles):
            for slot in range(nps):
                v = proc[c, nstart + slot]
                if v < 0:
                    continue   # dummy: S col has keep-alive rows anyway
                gv = int(assign[c][int(v)])
                e0 = starts[gv]
                p0 = slot * b
                esrc1[c, p0:p0 + b, s] = src[e0:e0 + b]
                esrc2[c, p0:p0 + b, s] = store[src[e0:e0 + b]]

    # dummy-slot rows gather row 0 (esrc already 0) and contribute to den
    # of the dummy node only; outputs for dummies are masked / ignored.

    xT = np.zeros((cfg.IN, cfg.NT_PAD), NPBF)
    xT[:, :N] = x.T.astype(NPBF)
    in_maps = []
    A1 = np.zeros((cfg.F1, HEADS), np.float32)
    for h in range(HEADS):
        A1[h * cfg.HID:(h + 1) * cfg.HID, h] = W["att1"][h]
    A2 = np.zeros((cfg.F2, HEADS), np.float32)
    for h in range(HEADS):
        A2[h * cfg.OUT:(h + 1) * cfg.OUT, h] = W["att2"][h]
    # xl table rhs: [Wl1 | 0.6*Wl1@A1blk]; logits matmul uses 0.4*A
    Wcat1 = np.concatenate([W["Wl1"], 0.6 * (W["Wl1"] @ A1)], 1).astype(np.float32)
    Wcat2 = np.concatenate([W["Wl2"], 0.6 * (W["Wl2"] @ A2)], 1).astype(np.float32)
    for c in range(NCORES):
        xTo = np.zeros((cfg.IN, NODES_PAD), NPBF)
        sel = proc[c] >= 0
        xTo[:, sel] = x[assign[c][proc[c][sel]]].T.astype(NPBF)
        mask32 = np.zeros((cfg.HID, NODES_PAD), NPBF)
        mask32[:, sel] = 1.0
        mean1m = np.zeros((128, cfg.HID), np.float32)
        mean1m[np.arange(128), np.arange(128) % cfg.HID] = 0.125
        mean2m = np.zeros((128, cfg.OUT), np.float32)
        mean2m[np.arange(128), np.arange(128) % cfg.OUT] = 0.125
        e1m = np.zeros((8, cfg.F1), np.float32)
        e1m[np.arange(cfg.F1) // cfg.HID, np.arange(cfg.F1)] = 1.0
        e2m = np.zeros((8, cfg.F2), np.float32)
        e2m[np.arange(cfg.F2) // cfg.OUT, np.arange(cfg.F2)] = 1.0
        in_maps.append({
            "xT": xT, "xTo": xTo, "mask32": mask32,
            "mean1m": mean1m, "mean2m": mean2m, "e1m": e1m, "e2m": e2m,
            "esrc1": np.ascontiguousarray(esrc1[c]),
            "esrc2": np.ascontiguousarray(esrc2[c]),
            "Stbl": S_np,
            "Wl1m": W["Wl1"].reshape(cfg.IN, HEADS, cfg.HID).mean(1).astype(np.float32),
            "Wl2m": W["Wl2"].reshape(cfg.HID, HEADS, cfg.OUT).mean(1).astype(np.float32),
            "Wcat1": Wcat1, "Wcat2": Wcat2,
            "Wr1": W["Wr1"].astype(np.float32),
            "Wr2": W["Wr2"].astype(np.float32),
            "A1": (0.4 * A1).astype(np.float32),
            "A2": (0.4 * A2).astype(np.float32),
            "gb1": np.stack([W["gamma1"], W["beta1"]], 1).astype(np.float32),
            "gb2": np.stack([W["gamma2"], W["beta2"]], 1).astype(np.float32),
            "b1c": W["b1"].reshape(-1, 1).astype(np.float32),
            "b2c": W["b2"].reshape(-1, 1).astype(np.float32),
        })
    meta = dict(NODES_PAD=NODES_PAD, NSUB=NSUB, NG=NG, subtiles=subtiles,
                proc=proc, assign=assign, in_maps=in_maps)
    return meta


def _build(cfg, meta):
    NODES_PAD, NSUB, NG = meta["NODES_PAD"], meta["NSUB"], meta["NG"]
    subtiles = meta["subtiles"]
    IN, F1, F2, HID, OUT = cfg.IN, cfg.F1, cfg.F2, cfg.HID, cfg.OUT
    F1E, F2E = cfg.F1E, cfg.F2E
    K0, K1 = cfg.KT
    NTP = cfg.NT_PAD
    C1, C2 = HID, OUT
    n_xl1_tiles = NTP // 128
    RECIP_N = 1.0 / cfg.N

    nc = bacc.Bacc("TRN2", target_bir_lowering=False, debug=False,
                   num_devices=NCORES)
    din = {}
    for name, shape, dt in [
            ("xT", [IN, NTP], BF), ("xTo", [IN, NODES_PAD], BF),
            ("mask32", [HID, NODES_PAD], BF),
            ("esrc1", [128, NSUB], I32), ("esrc2", [128, NSUB], I32),
            ("Stbl", [128, NODES_PAD], BF),
            ("Wcat1", [IN, F1E], F32), ("Wr1", [IN, F1], F32),
            ("Wcat2", [HID, F2E], F32), ("Wr2", [HID, F2], F32),
            ("A1", [F1, HEADS], F32), ("A2", [F2, HEADS], F32),
            ("gb1", [IN, 2], F32), ("gb2", [HID, 2], F32),
            ("b1c", [HID, 1], F32), ("b2c", [OUT, 1], F32),
            ("mean1m", [128, HID], F32), ("mean2m", [128, OUT], F32),
            ("Wl1m", [IN, HID], F32), ("Wl2m", [HID, OUT], F32),
            ("e1m", [8, F1], F32), ("e2m", [8, F2], F32)]:
        din[name] = nc.dram_tensor(name, shape, dt, kind="ExternalInput")
    outT = nc.dram_tensor("outT", [OUT, NODES_PAD], F32, kind="ExternalOutput")

    xl1_full = nc.dram_tensor("xl1_full", [NTP, F1E], BF)
    xl2_full = nc.dram_tensor("xl2_full", [NCORES * NODES_PAD, F2E], BF)
    st1_in = nc.dram_tensor("st1_in", [IN, 2], F32)
    bl1_d = nc.dram_tensor("bl1_d", [1, HID], F32)
    bl2_d = nc.dram_tensor("bl2_d", [1, OUT], F32)
    st1_out = nc.dram_tensor("st1_out", [IN, 2], F32)
    ag_in = nc.dram_tensor("ag_in", [HID + 2, NODES_PAD], BF)
    ag_out = nc.dram_tensor("ag_out", [NCORES * (HID + 2), NODES_PAD], BF,
                            addr_space="Shared")

    import contextlib
    with tile.TileContext(nc) as tc:
        ctx = contextlib.ExitStack()
        with ctx:
            cpool = ctx.enter_context(tc.tile_pool(name="const", bufs=1))
            rpool = ctx.enter_context(tc.tile_pool(name="resident", bufs=1))

            # ---------- constants ----------
            ident = cpool.tile([128, 128], BF)
            make_identity(nc, ident[:])
            ones_row = cpool.tile([1, 128], BF)
            nc.vector.memset(ones_row[:], 1.0)
            epsb = cpool.tile([128, 1], F32, tag="epsb")
            nc.vector.memset(epsb[:], BN_EPS)
            msh1 = cpool.tile([128, 1], F32, tag="msh1")
            nc.vector.memset(msh1[:], -float(cfg.M1))
            msh2 = cpool.tile([128, 1], F32, tag="msh2")
            nc.vector.memset(msh2[:], -float(cfg.M2))
            def const_bf(name, shape, tagn):
                tf = cpool.tile(shape, F32, tag=tagn + "f", name=tagn + "f")
                nc.sync.dma_start(out=tf[:], in_=din[name].ap())
                tb = cpool.tile(shape, BF, tag=tagn, name=tagn)
                nc.vector.tensor_copy(out=tb[:], in_=tf[:])
                return tb
            mean1 = const_bf("mean1m", [128, C1], "mean1")
            mean2 = const_bf("mean2m", [128, C2], "mean2")
            e1full = const_bf("e1m", [8, F1], "e1m")
            e2full = const_bf("e2m", [8, F2], "e2m")
            e1h = [e1full[:, h * 128:(h + 1) * 128] for h in range(F1 // 128)]
            e2h = [e2full[:, h * 128:(h + 1) * 128] for h in range(F2 // 128)]

            # index / one-hot tables resident
            esrc1_sb = rpool.tile([128, NSUB], I32)
            nc.sync.dma_start(out=esrc1_sb[:], in_=din["esrc1"].ap())
            esrc2_sb = rpool.tile([128, NSUB], I32)
            nc.sync.dma_start(out=esrc2_sb[:], in_=din["esrc2"].ap())
            S_sb = rpool.tile([128, NODES_PAD], BF)
            nc.sync.dma_start(out=S_sb[:], in_=din["Stbl"].ap())
            a1_sb = []
            for h in range(F1 // 128):
                t = rpool.tile([128, HEADS], BF, tag=f"a1_{h}")
                tf = rpool.tile([128, HEADS], F32, tag=f"a1f_{h}")
                nc.sync.dma_start(out=tf[:], in_=din["A1"].ap()[h * 128:(h + 1) * 128, :])
                nc.vector.tensor_copy(out=t[:], in_=tf[:])
                a1_sb.append(t)
            a2_sb = []
            for h in range(F2 // 128):
                t = rpool.tile([128, HEADS], BF, tag=f"a2_{h}")
                tf = rpool.tile([128, HEADS], F32, tag=f"a2f_{h}")
                nc.sync.dma_start(out=tf[:], in_=din["A2"].ap()[h * 128:(h + 1) * 128, :])
                nc.vector.tensor_copy(out=t[:], in_=tf[:])
                a2_sb.append(t)
            b1c_sb = rpool.tile([HID, 1], F32)
            nc.sync.dma_start(out=b1c_sb[:], in_=din["b1c"].ap())
            b2c_sb = rpool.tile([OUT, 1], F32)
            nc.sync.dma_start(out=b2c_sb[:], in_=din["b2c"].ap())
            mask32_sb = rpool.tile([HID, NODES_PAD], BF)
            nc.sync.dma_start(out=mask32_sb[:], in_=din["mask32"].ap())

            kt_sizes = [K0] + ([K1] if K1 else [])
            xTo_sb = []
            for ki, ks in enumerate(kt_sizes):
                t = rpool.tile([ks, NODES_PAD], BF, tag=f"xTo{ki}")
                nc.sync.dma_start(out=t[:], in_=din["xTo"].ap()[ki * 128:ki * 128 + ks, :])
                xTo_sb.append(t)

            # ---------- phase A: BN1 stats + AllReduce ----------
            with tc.tile_pool(name="pA", bufs=2) as pa, \
                 tc.tile_pool(name="pAs", bufs=1) as pas:
                for ki, ks in enumerate(kt_sizes):
                    st = pa.tile([ks, 2], F32, tag="st")
                    nc.vector.tensor_reduce(out=st[:, 0:1], in_=xTo_sb[ki][:],
                                            axis=mybir.AxisListType.X,
                                            op=mybir.AluOpType.add)
                    scr = pas.tile([ks, NODES_PAD], BF, tag="scr")
                    nc.scalar.activation(out=scr[:], in_=xTo_sb[ki][:],
                                         func=mybir.ActivationFunctionType.Square,
                                         accum_out=st[:, 1:2])
                    nc.sync.dma_start(out=st1_in.ap()[ki * 128:ki * 128 + ks, :],
                                      in_=st[:])
            nc.gpsimd.collective_compute(
                "AllReduce", mybir.AluOpType.add,
                ins=[st1_in.ap()], outs=[st1_out.ap()],
                replica_groups=[list(range(NCORES))])

            # fold stats -> s1, t1 and scaled weights
            s1_t, t1_t = [], []
            wl1s, wr1s = [], []
            with tc.tile_pool(name="pB", bufs=1) as pb:
                for ki, ks in enumerate(kt_sizes):
                    stg = pb.tile([ks, 2], F32, tag=f"stg{ki}")
                    nc.sync.dma_start(out=stg[:], in_=st1_out.ap()[ki * 128:ki * 128 + ks, :])
                    gb = pb.tile([ks, 2], F32, tag=f"gb{ki}")
                    nc.sync.dma_start(out=gb[:], in_=din["gb1"].ap()[ki * 128:ki * 128 + ks, :])
                    mean = pb.tile([ks, 1], F32, tag=f"mean{ki}")
                    nc.vector.tensor_scalar(out=mean[:], in0=stg[:, 0:1],
                                            scalar1=RECIP_N, scalar2=None,
                                            op0=mybir.AluOpType.mult)
                    q = pb.tile([ks, 1], F32, tag=f"q{ki}")
                    nc.vector.tensor_scalar(out=q[:], in0=stg[:, 1:2],
                                            scalar1=RECIP_N, scalar2=None,
                                            op0=mybir.AluOpType.mult)
                    m2 = pb.tile([ks, 1], F32, tag=f"m2{ki}")
                    nc.vector.tensor_tensor(out=m2[:], in0=mean[:], in1=mean[:],
                                            op=mybir.AluOpType.mult)
                    var = pb.tile([ks, 1], F32, tag=f"var{ki}")
                    nc.vector.tensor_tensor(out=var[:], in0=q[:], in1=m2[:],
                                            op=mybir.AluOpType.subtract)
                    sd = pb.tile([ks, 1], F32, tag=f"sd{ki}")
                    nc.scalar.activation(out=sd[:], in_=var[:],
                                         func=mybir.ActivationFunctionType.Sqrt,
                                         bias=epsb[:ks, :1])
                    rstd = pb.tile([ks, 1], F32, tag=f"rstd{ki}")
                    nc.vector.reciprocal(rstd[:], sd[:])
                    s1 = pb.tile([ks, 1], F32, tag=f"s1{ki}")
                    nc.vector.tensor_tensor(out=s1[:], in0=gb[:, 0:1], in1=rstd[:],
                                            op=mybir.AluOpType.mult)
                    ms = pb.tile([ks, 1], F32, tag=f"ms{ki}")
                    nc.vector.tensor_tensor(out=ms[:], in0=mean[:], in1=s1[:],
                                            op=mybir.AluOpType.mult)
                    t1 = pb.tile([ks, 1], F32, tag=f"t1{ki}")
                    nc.vector.tensor_tensor(out=t1[:], in0=gb[:, 1:2], in1=ms[:],
                                            op=mybir.AluOpType.subtract)
                    s1_t.append(s1)
                    t1_t.append(t1)
                    wcf = pb.tile([ks, F1E], F32, tag=f"wcf{ki}", name=f"wcf{ki}")
                    nc.sync.dma_start(out=wcf[:], in_=din["Wcat1"].ap()[ki * 128:ki * 128 + ks, :])
                    wcs = rpool.tile([ks, F1E], BF, tag=f"wcs{ki}")
                    nc.vector.tensor_scalar(out=wcs[:], in0=wcf[:],
                                            scalar1=s1[:, :1], scalar2=None,
                                            op0=mybir.AluOpType.mult)
                    wl1s.append(wcs)
                    wrf = pb.tile([ks, F1], F32, tag=f"wrf{ki}", name=f"wrf{ki}")
                    nc.sync.dma_start(out=wrf[:], in_=din["Wr1"].ap()[ki * 128:ki * 128 + ks, :])
                    wrs = rpool.tile([ks, F1], BF, tag=f"wrs{ki}")
                    nc.vector.tensor_scalar(out=wrs[:], in0=wrf[:],
                                            scalar1=s1[:, :1], scalar2=None,
                                            op0=mybir.AluOpType.mult)
                    wr1s.append(wrs)
                    # keep f32 sum for bias12
                    wsumf = pb.tile([ks, F1], F32, tag=f"wsumf{ki}",
                                    name=f"wsumf{ki}")
                    nc.vector.tensor_tensor(out=wsumf[:], in0=wcf[:, :F1],
                                            in1=wrf[:], op=mybir.AluOpType.add)
                    wsb = pb.tile([ks, F1], BF, tag=f"wsb{ki}", name=f"wsb{ki}")
                    nc.vector.tensor_copy(out=wsb[:], in_=wsumf[:])
                    if ki == 0:
                        wsum_t = [wsb]
                    else:
                        wsum_t.append(wsb)
                t1b = []
                for ki, ks in enumerate(kt_sizes):
                    tb = pb.tile([ks, 1], BF, tag=f"t1b{ki}")
                    nc.vector.tensor_copy(out=tb[:], in_=t1_t[ki][:])
                    t1b.append(tb)
                with tc.tile_pool(name="pBp", bufs=1, space="PSUM") as pbp:
                    bps = pbp.tile([1, F1], F32, space="PSUM")
                    for ki, ks in enumerate(kt_sizes):
                        nc.tensor.matmul(out=bps[:], lhsT=t1b[ki][:],
                                         rhs=wsum_t[ki][:],
                                         start=(ki == 0),
                                         stop=(ki == len(kt_sizes) - 1))
                    bias12 = rpool.tile([1, F1], BF)
                    nc.vector.tensor_copy(out=bias12[:], in_=bps[:])
                with tc.tile_pool(name="pBq", bufs=1, space="PSUM") as pbq:
                    blp = pbq.tile([1, HID], F32, space="PSUM")
                    for ki, ks in enumerate(kt_sizes):
                        wmf = pb.tile([ks, HID], F32, tag=f"wmf{ki}",
                                      name=f"wmf{ki}")
                        nc.sync.dma_start(
                            out=wmf[:],
                            in_=din["Wl1m"].ap()[ki * 128:ki * 128 + ks, :])
                        wmb = pb.tile([ks, HID], BF, tag=f"wmb{ki}",
                                      name=f"wmb{ki}")
                        nc.vector.tensor_copy(out=wmb[:], in_=wmf[:])
                        nc.tensor.matmul(out=blp[:], lhsT=t1b[ki][:], rhs=wmb[:],
                                         start=(ki == 0),
                                         stop=(ki == len(kt_sizes) - 1))
                    blr = pb.tile([1, HID], F32, tag="blr")
                    nc.vector.tensor_copy(out=blr[:], in_=blp[:])
                    nc.sync.dma_start(out=bl1_d.ap(), in_=blr[:])
                blc = rpool.tile([HID, 1], F32)
                nc.sync.dma_start(out=blc[:], in_=bl1_d.ap())
                bias1t = rpool.tile([HID, 1], F32)
                nc.vector.tensor_tensor(out=bias1t[:], in0=blc[:], in1=b1c_sb[:],
                                        op=mybir.AluOpType.add)

            # ---------- phase A2: xr1T resident (feat-major, own nodes) ----
            xr1T = []
            with tc.tile_pool(name="pC", bufs=2, space="PSUM") as pc:
                for g in range(NG):
                    halves = []
                    for h in range(F1 // 128):
                        ps = pc.tile([128, 128], F32, space="PSUM", tag="xr1p")
                        for ki, ks in enumerate(kt_sizes):
                            nc.tensor.matmul(
                                out=ps[:],
                                lhsT=wr1s[ki][:, h * 128:(h + 1) * 128],
                                rhs=xTo_sb[ki][:, g * 128:(g + 1) * 128],
                                start=(ki == 0), stop=False)
                        nc.tensor.matmul(out=ps[:],
                                         lhsT=bias12[:, h * 128:(h + 1) * 128],
                                         rhs=ones_row[:],
                                         start=False, stop=True)
                        t = rpool.tile([128, 128], BF, tag=f"xr1T_{g}_{h}")
                        if (g + h) % 2 == 0:
                            nc.vector.tensor_copy(out=t[:], in_=ps[:])
                        else:
                            nc.scalar.copy(out=t[:], in_=ps[:])
                        halves.append(t)
                    xr1T.append(halves)

            # ---------- phase A3: xl1_full table (chunked DMA) ----------
            CH = 28
            with tc.tile_pool(name="pD", bufs=2) as pd, \
                 tc.tile_pool(name="pDo", bufs=2) as pdo, \
                 tc.tile_pool(name="pDp", bufs=2, space="PSUM") as pdp:
                for c0 in range(0, n_xl1_tiles, CH):
                    nt = min(CH, n_xl1_tiles - c0)
                    lhs = []
                    for ki, ks in enumerate(kt_sizes):
                        lt = pd.tile([ks, CH * 128], BF, tag=f"xl1l{ki}")
                        nc.sync.dma_start(
                            out=lt[:, :nt * 128],
                            in_=din["xT"].ap()[ki * 128:ki * 128 + ks,
                                               c0 * 128:(c0 + nt) * 128])
                        lhs.append(lt)
                    ob = pdo.tile([128, CH * F1E], BF, tag="xl1o")
                    for j in range(nt):
                        ps = pdp.tile([128, F1E], F32, space="PSUM", tag="xl1p")
                        for ki, ks in enumerate(kt_sizes):
                            nc.tensor.matmul(
                                out=ps[:], lhsT=lhs[ki][:, j * 128:(j + 1) * 128],
                                rhs=wl1s[ki][:],
                                start=(ki == 0), stop=(ki == len(kt_sizes) - 1))
                        osl = ob[:, j * F1E:(j + 1) * F1E]
                        if j % 2 == 0:
                            nc.vector.tensor_copy(out=osl, in_=ps[:])
                        else:
                            nc.scalar.copy(out=osl, in_=ps[:])
                    nc.sync.dma_start(
                        out=xl1_full.ap()[c0 * 128:(c0 + nt) * 128, :]
                            .rearrange("(j p) f -> p j f", j=nt),
                        in_=ob[:, :nt * F1E].rearrange("p (j f) -> p j f", j=nt))

            # ---------- helper: edge phase ----------
            def edge_phase(F, FE, xfull, esrc_sb, ah_sb, eh_mats, meanm,
                           shift_ap, bias_col, out_cb, layer):
                nhalf = F // 128
                NB = 4 if nhalf == 2 else 8      # subtiles per batch
                groups = {}
                for s, (b, nstart, nps) in enumerate(subtiles):
                    groups.setdefault(nstart // 128, []).append(
                        (s, b, nstart % 128, nps))
                with tc.tile_pool(name=f"ge{layer}", bufs=6) as gp, \
                     tc.tile_pool(name=f"gz{layer}", bufs=2, space="PSUM") as gz, \
                     tc.tile_pool(name=f"gl{layer}", bufs=2, space="PSUM") as gl, \
                     tc.tile_pool(name=f"gn{layer}", bufs=2, space="PSUM") as gn, \
                     tc.tile_pool(name=f"gs{layer}", bufs=4) as gs:
                    for g in range(NG):
                        subs = groups[g]
                        nd = gn.tile([128, nhalf * 128 + 128], F32,
                                     space="PSUM", tag="numT")
                        numT = nd[:, :nhalf * 128]
                        denT = nd[:8, nhalf * 128:]
                        for b0 in range(0, len(subs), NB):
                            batch = subs[b0:b0 + NB]
                            nb = len(batch)
                            # z layout: column block (si*nhalf+h)*128
                            zts = gz.tile([128, nb * nhalf * 128], F32,
                                          space="PSUM", tag="zt", name="zt")
                            lg = gl.tile([128, nb * 8], F32, space="PSUM", tag="lg")
                            xls4 = gp.tile([128, NB * FE], BF, tag="xls")
                            for si, (s, b, noff, nps) in enumerate(batch):
                                nc.gpsimd.indirect_dma_start(
                                    out=xls4[:, si * FE:(si + 1) * FE],
                                    out_offset=None,
                                    in_=xfull.ap(),
                                    in_offset=bass.IndirectOffsetOnAxis(
                                        ap=esrc_sb[:, s:s + 1], axis=0))
                                for h in range(nhalf):
                                    zsl = zts[:, (si * nhalf + h) * 128:
                                              (si * nhalf + h + 1) * 128]
                                    nc.tensor.matmul(
                                        out=zsl,
                                        lhsT=xls4[:, si * FE + h * 128:
                                                  si * FE + (h + 1) * 128],
                                        rhs=ident[:],
                                        start=True, stop=False)
                                    xr_ap = (xr1T[g][h] if layer == 1 else xr2T[g])
                                    rep = xr_ap[:, noff:noff + nps, None] \
                                        .broadcast_to([128, nps, b])
                                    nc.tensor.matmul(
                                        out=zsl[:, :nps * b], lhsT=ident[:],
                                        rhs=rep, start=False, stop=True)
                                    if nps * b < 128:
                                        rep2 = xr_ap[:, noff:noff + 1, None] \
                                            .broadcast_to([128, 1, 128 - nps * b])
                                        nc.tensor.matmul(
                                            out=zsl[:, nps * b:],
                                            lhsT=ident[:],
                                            rhs=rep2, start=False, stop=True)
                            es = gs.tile([128, nb * nhalf * 128], BF, tag="es",
                                         name="es")
                            nc.scalar.activation(
                                out=es[:], in_=zts[:],
                                func=mybir.ActivationFunctionType.Abs)
                            for si, (s, b, noff, nps) in enumerate(batch):
                                lsl = lg[:, si * 8:(si + 1) * 8]
                                for h in range(nhalf):
                                    nc.tensor.matmul(
                                        out=lsl,
                                        lhsT=es[:, (si * nhalf + h) * 128:
                                                (si * nhalf + h + 1) * 128],
                                        rhs=ah_sb[h][:],
                                        start=(h == 0), stop=False)
                                nc.tensor.matmul(
                                    out=lsl, lhsT=ident[:],
                                    rhs=xls4[:, si * FE + F:si * FE + F + 8],
                                    start=False, stop=True)
                            w4 = gs.tile([128, nb * 8], BF, tag="w4")
                            nc.scalar.activation(
                                out=w4[:], in_=lg[:],
                                func=mybir.ActivationFunctionType.Exp,
                                bias=shift_ap[:, :1])
                            y4 = gp.tile([128, NB * F], BF, tag="y")
                            xls_f = xls4[:, :nb * FE].rearrange(
                                "p (s f) -> p s f", s=nb)[:, :, :F] \
                                .rearrange("p s (a b) -> p s a b", a=8)
                            wv = w4[:, :nb * 8, None].rearrange(
                                "p (s a) b -> p s a b", s=nb) \
                                .broadcast_to([128, nb, 8, F // 8])
                            nc.vector.tensor_tensor(
                                out=y4[:, :nb * F].rearrange(
                                    "p (s a b) -> p s a b", s=nb, a=8),
                                in0=xls_f, in1=wv, op=mybir.AluOpType.mult)
                            for si, (s, b, noff, nps) in enumerate(batch):
                                S_ap = S_sb[:, g * 128 + noff:g * 128 + noff + nps]
                                for h in range(nhalf):
                                    nc.tensor.matmul(
                                        out=numT[:, h * 128 + noff:h * 128 + noff + nps],
                                        lhsT=y4[:, si * F + h * 128:
                                                si * F + (h + 1) * 128],
                                        rhs=S_ap, start=True, stop=True)
                                nc.tensor.matmul(
                                    out=denT[0:8, noff:noff + nps],
                                    lhsT=w4[:, si * 8:(si + 1) * 8],
                                    rhs=S_ap, start=True, stop=True)
                        # ---- group epilogue ----
                        drec = gs.tile([8, 128], F32, tag="drec")
                        nc.vector.reciprocal(drec[:], denT[:])
                        drecb = gs.tile([8, 128], BF, tag="drecb")
                        nc.vector.tensor_copy(out=drecb[:], in_=drec[:])
                        onts = []
                        for h in range(nhalf):
                            rexp = gz.tile([128, 128], F32, space="PSUM",
                                           tag="zt")
                            nc.tensor.matmul(out=rexp[:], lhsT=eh_mats[h],
                                             rhs=drecb[:], start=True, stop=True)
                            rexpb = gs.tile([128, 128], BF, tag=f"rexpb{h}",
                                            name=f"rexpb{h}")
                            nc.scalar.copy(out=rexpb[:], in_=rexp[:])
                            ont = gs.tile([128, 128], BF, tag=f"ont{h}",
                                          name=f"ont{h}")
                            nc.vector.tensor_tensor(
                                out=ont[:], in0=numT[:, h * 128:(h + 1) * 128],
                                in1=rexpb[:], op=mybir.AluOpType.mult)
                            onts.append(ont)
                        cdim = C1 if layer == 1 else C2
                        ot = gl.tile([cdim, 128], F32, space="PSUM", tag="lg")
                        for h in range(nhalf):
                            nc.tensor.matmul(out=ot[:], lhsT=meanm[:, :cdim],
                                             rhs=onts[h][:], start=(h == 0),
                                             stop=(h == nhalf - 1))
                        out_cb(g, ot, bias_col)

            # ---------- phase B: layer-1 edges -> h1T ----------
            h1T = rpool.tile([HID, NODES_PAD], BF)
            oB = ctx.enter_context(tc.tile_pool(name="oB", bufs=2))

            def l1_out(g, ot_psum, bias_col):
                hrel = oB.tile([HID, 128], BF, tag="hrel")
                nc.scalar.activation(out=hrel[:], in_=ot_psum[:],
                                     func=mybir.ActivationFunctionType.Relu,
                                     bias=bias_col[:, :1])
                nc.vector.tensor_tensor(out=h1T[:, g * 128:(g + 1) * 128],
                                        in0=hrel[:],
                                        in1=mask32_sb[:, g * 128:(g + 1) * 128],
                                        op=mybir.AluOpType.mult)

            edge_phase(F1, F1E, xl1_full, esrc1_sb, a1_sb, e1h, mean1, msh1,
                       bias1t, l1_out, layer=1)

            # ---------- phase C: AllGather h1T + BN2 + xl2 + xr2T ----------
            with tc.tile_pool(name="pE", bufs=2) as pe:
                st2 = pe.tile([HID, 2], F32, tag="st2")
                nc.vector.tensor_reduce(out=st2[:, 0:1], in_=h1T[:],
                                        axis=mybir.AxisListType.X,
                                        op=mybir.AluOpType.add)
                scr2 = pe.tile([HID, NODES_PAD], BF, tag="scr2")
                nc.scalar.activation(out=scr2[:], in_=h1T[:],
                                     func=mybir.ActivationFunctionType.Square,
                                     accum_out=st2[:, 1:2])
                nc.sync.dma_start(out=ag_in.ap()[0:HID, :], in_=h1T[:])
                nc.sync.dma_start(out=ag_in.ap()[HID:HID + 1, 0:2 * HID],
                                  in_=st2[:, 0:1].bitcast(BF))
                nc.sync.dma_start(out=ag_in.ap()[HID + 1:HID + 2, 0:2 * HID],
                                  in_=st2[:, 1:2].bitcast(BF))
            nc.gpsimd.collective_compute(
                "AllGather", mybir.AluOpType.bypass,
                ins=[ag_in.ap()], outs=[ag_out.ap()],
                replica_groups=[list(range(NCORES))])

            with tc.tile_pool(name="pF", bufs=1) as pf:
                s2sum = pf.tile([HID, NCORES], F32, tag="s2sum")
                s2sq = pf.tile([HID, NCORES], F32, tag="s2sq")
                agf = ag_out.ap().bitcast(F32)
                for c in range(NCORES):
                    r = c * (HID + 2) + HID
                    nc.sync.dma_start(out=s2sum[:, c:c + 1],
                                      in_=agf[r:r + 1, 0:HID])
                    nc.sync.dma_start(out=s2sq[:, c:c + 1],
                                      in_=agf[r + 1:r + 2, 0:HID])
                stg = pf.tile([HID, 2], F32, tag="stg2")
                nc.vector.tensor_reduce(out=stg[:, 0:1], in_=s2sum[:],
                                        axis=mybir.AxisListType.X,
                                        op=mybir.AluOpType.add)
                nc.vector.tensor_reduce(out=stg[:, 1:2], in_=s2sq[:],
                                        axis=mybir.AxisListType.X,
                                        op=mybir.AluOpType.add)
                gb = pf.tile([HID, 2], F32, tag="gb2")
                nc.sync.dma_start(out=gb[:], in_=din["gb2"].ap())
                mean = pf.tile([HID, 1], F32, tag="mean2")
                nc.vector.tensor_scalar(out=mean[:], in0=stg[:, 0:1],
                                        scalar1=RECIP_N, scalar2=None,
                                        op0=mybir.AluOpType.mult)
                q = pf.tile([HID, 1], F32, tag="q2")
                nc.vector.tensor_scalar(out=q[:], in0=stg[:, 1:2],
                                        scalar1=RECIP_N, scalar2=None,
                                        op0=mybir.AluOpType.mult)
                m2 = pf.tile([HID, 1], F32, tag="m22")
                nc.vector.tensor_tensor(out=m2[:], in0=mean[:], in1=mean[:],
                                        op=mybir.AluOpType.mult)
                var = pf.tile([HID, 1], F32, tag="var2")
                nc.vector.tensor_tensor(out=var[:], in0=q[:], in1=m2[:],
                                        op=mybir.AluOpType.subtract)
                sd = pf.tile([HID, 1], F32, tag="sd2")
                nc.scalar.activation(out=sd[:], in_=var[:],
                                     func=mybir.ActivationFunctionType.Sqrt,
                                     bias=epsb[:HID, :1])
                rstd = pf.tile([HID, 1], F32, tag="rstd2")
                nc.vector.reciprocal(rstd[:], sd[:])
                s2 = pf.tile([HID, 1], F32, tag="s2")
                nc.vector.tensor_tensor(out=s2[:], in0=gb[:, 0:1], in1=rstd[:],
                                        op=mybir.AluOpType.mult)
                ms = pf.tile([HID, 1], F32, tag="ms2")
                nc.vector.tensor_tensor(out=ms[:], in0=mean[:], in1=s2[:],
                                        op=mybir.AluOpType.mult)
                t2 = pf.tile([HID, 1], F32, tag="t2")
                nc.vector.tensor_tensor(out=t2[:], in0=gb[:, 1:2], in1=ms[:],
                                        op=mybir.AluOpType.subtract)
                wc2f = pf.tile([HID, F2E], F32, tag="wc2f")
                nc.sync.dma_start(out=wc2f[:], in_=din["Wcat2"].ap())
                wr2f = pf.tile([HID, F2], F32, tag="wr2f")
                nc.sync.dma_start(out=wr2f[:], in_=din["Wr2"].ap())
                wl2s = rpool.tile([HID, F2E], BF)
                nc.vector.tensor_scalar(out=wl2s[:], in0=wc2f[:],
                                        scalar1=s2[:, :1], scalar2=None,
                                        op0=mybir.AluOpType.mult)
                wr2s = rpool.tile([HID, F2], BF)
                nc.vector.tensor_scalar(out=wr2s[:], in0=wr2f[:],
                                        scalar1=s2[:, :1], scalar2=None,
                                        op0=mybir.AluOpType.mult)
                t2b = pf.tile([HID, 1], BF, tag="t2b")
                nc.vector.tensor_copy(out=t2b[:], in_=t2[:])
                wsum = pf.tile([HID, F2], BF, tag="wsum")
                nc.vector.tensor_tensor(out=wsum[:], in0=wc2f[:, :F2],
                                        in1=wr2f[:], op=mybir.AluOpType.add)
                with tc.tile_pool(name="pFp", bufs=1, space="PSUM") as pfp:
                    bps = pfp.tile([1, F2], F32, space="PSUM")
                    nc.tensor.matmul(out=bps[:], lhsT=t2b[:], rhs=wsum[:],
                                     start=True, stop=True)
                    bias22 = rpool.tile([1, F2], BF)
                    nc.vector.tensor_copy(out=bias22[:], in_=bps[:])
                with tc.tile_pool(name="pFq", bufs=1, space="PSUM") as pfq:
                    wmf2 = pf.tile([HID, OUT], F32, tag="wmf2")
                    nc.sync.dma_start(out=wmf2[:], in_=din["Wl2m"].ap())
                    wmb2 = pf.tile([HID, OUT], BF, tag="wmb2")
                    nc.vector.tensor_copy(out=wmb2[:], in_=wmf2[:])
                    blp2 = pfq.tile([1, OUT], F32, space="PSUM")
                    nc.tensor.matmul(out=blp2[:], lhsT=t2b[:], rhs=wmb2[:],
                                     start=True, stop=True)
                    blr2 = pf.tile([1, OUT], F32, tag="blr2")
                    nc.vector.tensor_copy(out=blr2[:], in_=blp2[:])
                    nc.sync.dma_start(out=bl2_d.ap(), in_=blr2[:])
                blc2 = rpool.tile([OUT, 1], F32)
                nc.sync.dma_start(out=blc2[:], in_=bl2_d.ap())
                bias2t = rpool.tile([OUT, 1], F32)
                nc.vector.tensor_tensor(out=bias2t[:], in0=blc2[:], in1=b2c_sb[:],
                                        op=mybir.AluOpType.add)

            # xl2_full (chunked per source-core stripe)
            CH2 = 25
            with tc.tile_pool(name="pG", bufs=2) as pg, \
                 tc.tile_pool(name="pGo", bufs=2) as pgo, \
                 tc.tile_pool(name="pGp", bufs=2, space="PSUM") as pgp:
                for c_src in range(NCORES):
                    for t0 in range(0, NG, CH2):
                        nt = min(CH2, NG - t0)
                        lhs2 = pg.tile([HID, CH2 * 128], BF, tag="xl2l")
                        nc.sync.dma_start(
                            out=lhs2[:, :nt * 128],
                            in_=ag_out.ap()[c_src * (HID + 2):c_src * (HID + 2) + HID,
                                            t0 * 128:(t0 + nt) * 128])
                        ob = pgo.tile([128, CH2 * F2E], BF, tag="xl2o")
                        for j in range(nt):
                            ps = pgp.tile([128, F2E], F32, space="PSUM", tag="xl2p")
                            nc.tensor.matmul(out=ps[:],
                                             lhsT=lhs2[:, j * 128:(j + 1) * 128],
                                             rhs=wl2s[:], start=True, stop=True)
                            osl = ob[:, j * F2E:(j + 1) * F2E]
                            if j % 2 == 0:
                                nc.vector.tensor_copy(out=osl, in_=ps[:])
                            else:
                                nc.scalar.copy(out=osl, in_=ps[:])
                        r0 = c_src * NODES_PAD + t0 * 128
                        nc.sync.dma_start(
                            out=xl2_full.ap()[r0:r0 + nt * 128, :]
                                .rearrange("(j p) f -> p j f", j=nt),
                            in_=ob[:, :nt * F2E].rearrange("p (j f) -> p j f",
                                                           j=nt))
            # xr2T resident
            xr2T = []
            with tc.tile_pool(name="pH", bufs=2, space="PSUM") as ph:
                for g in range(NG):
                    ps = ph.tile([128, 128], F32, space="PSUM", tag="xr2p")
                    nc.tensor.matmul(out=ps[:], lhsT=wr2s[:],
                                     rhs=h1T[:, g * 128:(g + 1) * 128],
                                     start=True, stop=False)
                    nc.tensor.matmul(out=ps[:], lhsT=bias22[:], rhs=ones_row[:],
                                     start=False, stop=True)
                    t = rpool.tile([128, 128], BF, tag=f"xr2T_{g}")
                    if g % 2 == 0:
                        nc.vector.tensor_copy(out=t[:], in_=ps[:])
                    else:
                        nc.scalar.copy(out=t[:], in_=ps[:])
                    xr2T.append(t)

            # ---------- phase D: layer-2 edges -> outT ----------
            oD = ctx.enter_context(tc.tile_pool(name="oD", bufs=2))

            def l2_out(g, ot_psum, bias_col):
                ob = oD.tile([OUT, 128], F32, tag="ob")
                nc.scalar.activation(out=ob[:], in_=ot_psum[:],
                                     func=mybir.ActivationFunctionType.Identity,
                                     bias=bias_col[:, :1])
                nc.sync.dma_start(out=outT.ap()[:, g * 128:(g + 1) * 128],
                                  in_=ob[:])

            edge_phase(F2, F2E, xl2_full, esrc2_sb, a2_sb, e2h, mean2, msh2,
                       bias2t, l2_out, layer=2)

    nc.compile()
    return nc


_CACHE = {}


def _get_nc(cfg, meta):
    key = (cfg.N, cfg.IN, cfg.HID, cfg.OUT, meta["NSUB"], meta["NODES_PAD"],
           tuple(meta["subtiles"]))
    if key not in _CACHE:
        _CACHE[key] = _build(cfg, meta)
    return _CACHE[key]


def run(cfg, inputs):
    x = np.asarray(inputs["x"], np.float32)
    ei = np.asarray(inputs["edge_index"], np.int32)
    W = {k: np.asarray(inputs[k], np.float32) for k in
         ("Wl1", "Wr1", "att1", "b1", "gamma1", "beta1",
          "Wl2", "Wr2", "att2", "b2", "gamma2", "beta2")}
    meta = _preprocess(cfg, x, ei, W)
    nc = _get_nc(cfg, meta)
    res = run_bass_kernel_spmd(nc, meta["in_maps"], core_ids=list(range(NCORES)))
    out = np.empty((cfg.N, cfg.OUT), np.float32)
    proc = meta["proc"]
    for c in range(NCORES):
        oT = res.results[c]["outT"]      # [OUT, NODES_PAD]
        sel = proc[c] >= 0
        out[meta["assign"][c][proc[c][sel]]] = oT[:, sel].T
    return out, meta, nc


def kernel(**inputs):
    cfg = Cfg(50000, 200, 32, 16, m1=8.0, m2=10.0)
    out, _, _ = run(cfg, inputs)
    return out
